# revision 74
# baseline (speedup 1.0000x reference)
"""Two-layer GCN on 8 Trainium2 NeuronCores — v2.1 (descriptor-lean).

HW is SWDGE-descriptor-rate bound (~4-5ns/desc at 4 queues, size-independent),
so v2.1 minimizes descriptor COUNT beyond the one-desc-per-edge gather:
- Self-loops never gathered: own-shard h1p/t2 terms are bulk-loaded p-major
  (~128 descs) and added on DVE.
- Per-pass partials scatter-add (f32, dense 256B rows) into a p-major
  accumulator keyed by true destination, so the epilogue re-reads the whole
  accumulator with 128 descriptors (one contiguous run per partition).
- All launch inputs/outputs that are per-destination use the plain p-major
  [128, 98*F] layout (dst = n*128 + p at [p, n*F:(n+1)*F]) — bulk DMA.
Layer tables as in v2: layer-1 bf16 pair-packed rows (128B descs via the
256B-stride sub-row gather), layer-2 f32 2-wide (8B descs). Gather perms are
per-pass global degree sorts (tight ~5% slot padding).
"""
import numpy as np

N = 100000
E_CH = 128
HID = 64
OUT = 2
NC = 8
PERCORE = 12500
PC_PAD = 12544
NCOLS = 98               # p-major columns (dst = n*128+p, n in [0,98))
NPASS = 4
L1_ROWS = 25001
L2_ROWS = 25001
ZROW = 25000
MAXPOS = 16384           # gather positions per call (slot-major k-blocks);
                         # stage tile is [128, (MAXPOS//128)*F]
STAGE_COLS = MAXPOS // 128


def _wrap_idx(vals):
    ni = len(vals)
    assert ni % 16 == 0
    return np.tile(vals.reshape(ni // 16, 16).T, (8, 1))


def _prep_layer(src, dst, pass_of, row_of):
    """Per-layer prep: per-pass degree-sorted perms, per-tile shared-K slot
    matrices, slot-major (k-block) gather calls, gather idx, and p-major
    scatter idx. No self-loops here.

    Slot-major: positions are ordered (k, tile, p) with c_k = #tiles whose
    K exceeds k. Each call covers consecutive k's with sum(c_k)*128 <=
    MAXPOS, so slot padding is per-tile-max only (~3%) and the reduce is a
    prefix-add tree on DVE (packed bf16/f32, 2x mode) instead of a strided
    tensor_reduce."""
    p = pass_of(src)
    r = row_of(src)
    core = dst // PERCORE
    dloc = dst % PERCORE

    cnt = np.bincount(dst * NPASS + p, minlength=N * NPASS).reshape(N, NPASS)

    perms = np.zeros((NC, NPASS, PC_PAD), np.int64)
    K = np.zeros((NPASS, PC_PAD // 128), np.int64)
    for i in range(NC):
        c0 = cnt[i * PERCORE:(i + 1) * PERCORE]
        for q in range(NPASS):
            pp = np.argsort(-c0[:, q], kind="stable")
            pp = np.concatenate([pp, np.arange(PERCORE, PC_PAD)])
            perms[i, q] = pp
            cc = np.concatenate([c0[:, q], np.zeros(PC_PAD - PERCORE, np.int64)])
            kt = cc[pp].reshape(-1, 128).max(axis=1)
            K[q] = np.maximum(K[q], kt)
    K = np.maximum(K, 1)

    invperms = np.zeros((NC, NPASS, PC_PAD), np.int64)
    for i in range(NC):
        for q in range(NPASS):
            invperms[i, q][perms[i, q]] = np.arange(PC_PAD)
    Ms = [[None] * NPASS for _ in range(NC)]
    for i in range(NC):
        ec = core == i
        for q in range(NPASS):
            sel = ec & (p == q)
            ed = dloc[sel]
            es = r[sel]
            order = np.argsort(ed, kind="stable")
            ed = ed[order]
            es = es[order]
            starts = np.searchsorted(ed, np.arange(PERCORE))
            rank = np.arange(len(ed)) - starts[ed]
            kmax = int(K[q].max())
            M = np.full((PC_PAD, kmax), ZROW, np.int16)
            M[invperms[i, q][ed], rank] = es.astype(np.int16)
            Ms[i][q] = M

    # slot-major call packing: per pass, greedy k-blocks under MAXPOS
    calls = []  # (pass, [c_k, ...]) with the k's implicit (sequential per pass)
    for q in range(NPASS):
        kmax = int(K[q].max())
        cs_all = [int((K[q] > k).sum()) for k in range(kmax)]
        cur = []
        for ck in cs_all:
            if cur and (sum(cur) + ck) * 128 > MAXPOS:
                calls.append((q, cur))
                cur = []
            cur.append(ck)
        if cur:
            calls.append((q, cur))

    idxg = []
    for i in range(NC):
        parts = []
        kpos = [0] * NPASS
        for (q, cs) in calls:
            k0 = kpos[q]
            vals = np.concatenate(
                [Ms[i][q][:ck * 128, k0 + j].reshape(ck, 128)
                 for j, ck in enumerate(cs)], axis=0)
            kpos[q] += len(cs)
            parts.append(_wrap_idx(vals.ravel()))
        idxg.append(np.concatenate(parts, axis=1))
    idxg = np.stack(idxg)  # [NC, 128, COLS_G]

    # scatter idx per (core, pass): perm position i -> p-major acc row of the
    # true dst; pad positions (perm rank >= PERCORE, trailing) -> -1
    idxsc = np.zeros((NC, NPASS, 128, PC_PAD // 16), np.int16)
    for i in range(NC):
        for q in range(NPASS):
            d = perms[i, q]
            v = ((d % 128) * NCOLS + d // 128).astype(np.int16)
            v[PERCORE:] = -1
            idxsc[i, q] = _wrap_idx(v)

    ndesc = int(K.sum()) * 128
    return calls, idxg, idxsc, ndesc


def _host_prep(edge_index):
    src = np.asarray(edge_index[0], dtype=np.int64)
    dst = np.asarray(edge_index[1], dtype=np.int64)
    deg = np.bincount(dst, minlength=N).astype(np.float64) + 1.0
    dis = (1.0 / np.sqrt(deg)).astype(np.float32)

    # no appended self-loops; added directly in the epilogues
    l1 = _prep_layer(src, dst,
                     lambda s: (s // 50000) * 2 + (s % 2),
                     lambda s: (s % 50000) // 2)
    l2 = _prep_layer(src, dst,
                     lambda s: s // 25000,
                     lambda s: s % 25000)
    return dis, l1, l2


def _bass_mods():
    import sys
    if "/opt/trn_rl_repo" not in sys.path:
        sys.path.insert(0, "/opt/trn_rl_repo")
    import concourse.bass as bass
    import concourse.bacc as bacc
    import concourse.tile as tile
    from concourse import mybir
    from concourse.bass_utils import run_bass_kernel_spmd
    return bass, bacc, tile, mybir, run_bass_kernel_spmd


def _dma_gather_thin(gp, out_ap, in_ap, idxs_ap, num_idxs, elem_size,
                     elem_step, queue_num):
    from concourse import mybir
    gp._assert_queue_num(queue_num)
    assert idxs_ap.dtype == mybir.dt.int16
    stride_bytes = elem_step * mybir.dt.size(in_ap.dtype)
    assert stride_bytes % 256 == 0 and stride_bytes // 256 < 256
    assert in_ap.ap[-1][1] == elem_size
    assert in_ap.ap[0][0] == elem_step
    _in_ap = gp.lower_ap_dma(in_ap, for_custom_bir_dma=True)
    _idxs_ap = gp.lower_ap(idxs_ap)
    _out_ap = gp.lower_ap(out_ap)
    return gp.add_instruction(
        mybir.InstDMAGatherAnt(
            name=gp.bass.get_next_instruction_name(),
            ins=[*_in_ap, _idxs_ap, gp.lower_val_access(gp.to_reg(num_idxs))],
            outs=[_out_ap],
            transpose=False,
            num_idxs=num_idxs,
            elem_size=elem_size,
            stride_bytes_256=stride_bytes // 256,
            gen_mode=0,
            single_packet=False,
            queue_num=queue_num,
            sbuf_tokens_per_rank=0,
            sbuf_free_dim_per_rank=0,
            sbuf_free_dim_pad_per_rank=0,
            sbuf_byte_offset=0,
        )
    )


def _build_mm():
    """h1p = (x @ W1) * dis for own shard, bf16, p-major output."""
    bass, bacc, tile, mybir, _ = _bass_mods()
    from contextlib import ExitStack
    nc = bacc.Bacc()
    bf = mybir.dt.bfloat16
    xT = nc.declare_dram_parameter("xT", [E_CH, PC_PAD], bf, isOutput=False)
    W1 = nc.declare_dram_parameter("W1", [E_CH, HID], bf, isOutput=False)
    disp = nc.declare_dram_parameter("disp", [128, NCOLS], mybir.dt.float32,
                                     isOutput=False)
    out = nc.declare_dram_parameter("out", [128, NCOLS * HID], bf,
                                    isOutput=True)
    G = 14
    with tile.TileContext(nc) as tc, ExitStack() as ctx:
        wp = ctx.enter_context(tc.tile_pool(name="wp", bufs=1))
        sb = ctx.enter_context(tc.tile_pool(name="sb", bufs=3))
        ps = ctx.enter_context(tc.tile_pool(name="ps", bufs=4, space="PSUM"))
        w1 = wp.tile([E_CH, HID], bf, tag="w1")
        nc.sync.dma_start(out=w1[:], in_=W1[:, :])
        dis_sb = wp.tile([128, NCOLS], mybir.dt.float32, tag="dis")
        nc.sync.dma_start(out=dis_sb[:], in_=disp[:, :])
        PB = 7
        for g in range(0, NCOLS, G):
            ng = min(G, NCOLS - g)
            xt = sb.tile([E_CH, G * 128], bf, tag="xt")
            nc.sync.dma_start(out=xt[:, :ng * 128],
                              in_=xT[:, g * 128:(g + ng) * 128])
            ot = sb.tile([128, G * HID], bf, tag="ot")
            for h0 in range(0, ng, PB):
                nh = min(PB, ng - h0)
                pt = ps.tile([128, PB * HID], mybir.dt.float32, space="PSUM",
                             tag="pt")
                for j in range(nh):
                    nc.tensor.matmul(pt[:, j * HID:(j + 1) * HID],
                                     lhsT=xt[:, (h0 + j) * 128:
                                             (h0 + j + 1) * 128],
                                     rhs=w1[:], start=True, stop=True)
                # scale each column's HID block by its dis in one strided op
                dview = bass.AP(dis_sb.tensor, dis_sb[:].offset + g + h0,
                                [dis_sb[:].ap[0], [1, nh], [0, HID]])
                pv = bass.AP(pt.tensor, pt[:].offset,
                             [pt[:].ap[0], [HID, nh], [1, HID]])
                ov = bass.AP(ot.tensor, ot[:].offset + h0 * HID,
                             [ot[:].ap[0], [HID, nh], [1, HID]])
                with nc.allow_low_precision(reason="bf16 h1 staging"):
                    nc.vector.tensor_tensor(out=ov, in0=pv, in1=dview,
                                            op=mybir.AluOpType.mult)
            nc.sync.dma_start(out=out[:, g * HID:(g + ng) * HID],
                              in_=ot[:, :ng * HID])
    nc.compile()
    return nc


def _common_agg(nc, bass, tile, mybir, ctx, tc, calls, tabs, idx0g, idxh,
                repb, idxsc, F, stage_dt, gather_elem, gather_step,
                tab_col_of, acc, acc_step, NQ, split_scatter=True,
                no_scatter=False, no_reduce=False, astrip_bufs=2,
                stage_bufs=4):
    """Shared gather/reduce/scatter pipeline. astrip is compact
    [128, NCOLS*F] in stage_dt; scatter writes F elems per destination into
    `acc` (row stride acc_step elems = 256B; untouched columns stay zero via
    output zero-donation).

    Gather indices arrive as hi/lo bf16 [32, cols] (idxh) and are broadcast
    to the q7-required 8x-replicated int16 [128, cols] layout on-chip:
    PE matmul against repb (256*rep | rep) then an exact f32->int16 convert
    on DVE. This cuts idx HBM traffic 4x. Call 0 uses a small direct int16
    load (idx0g) so the first gather isn't gated on the broadcast pipeline."""
    ib = ctx.enter_context(tc.tile_pool(name="ib", bufs=2))
    stp = ctx.enter_context(tc.tile_pool(name="stp", bufs=stage_bufs))
    ap_ = ctx.enter_context(tc.tile_pool(name="ap", bufs=astrip_bufs))
    psp = ctx.enter_context(tc.tile_pool(name="psp", bufs=2, space="PSUM"))

    cst_local = ctx.enter_context(tc.tile_pool(name="cstl", bufs=1))
    SCC = PC_PAD // 16
    iscb = cst_local.tile([128, NPASS * SCC], mybir.dt.int16, tag="iscb")
    iscb_loaded = [False]
    repb_sb = cst_local.tile([32, 128], mybir.dt.bfloat16, tag="repb")
    nc.sync.dma_start(out=repb_sb[:], in_=repb[:, :])
    BCH = 512  # psum-chunk columns per broadcast matmul

    def ensure_iscb():
        # deferred so the launch ramp isn't spent on scatter indices
        if not iscb_loaded[0]:
            nc.sync.dma_start(out=iscb[:], in_=idxsc[:, :])
            iscb_loaded[0] = True

    qn = 0
    goff = 0
    cur_pass = -1
    idx_sb = None
    idx0_sb = None
    pass_goff = 0
    astrip = None
    pass_cols = {}
    pass_ncalls = {}
    for (q, cs) in calls:
        pass_cols[q] = pass_cols.get(q, 0) + sum(cs) * 8
        pass_ncalls[q] = pass_ncalls.get(q, 0) + 1

    HCOL = NCOLS // 2          # 49 astrip columns per scatter half
    HPOS = HCOL * 128          # 6272 positions per half

    # per pass: index (within the pass) of the last call touching any tile
    # >= HCOL; after it, astrip cols [HCOL, NCOLS) are final (c_k shrinks)
    last_big = {}
    seen = {}
    for (q, cs) in calls:
        j = seen.get(q, 0)
        if cs[0] > HCOL:
            last_big[q] = j
        seen[q] = j + 1

    def flush_half(q, astrip_t, half):
        if no_scatter:
            return
        ensure_iscb()
        base = astrip_t[:]
        if not split_scatter and half == 1:
            nc.gpsimd.dma_scatter_add(
                out_ap=acc[:, :F],
                in_ap=astrip_t[:].rearrange("p (k f) -> p k f", k=NCOLS),
                idxs_ap=iscb[:, q * SCC:(q + 1) * SCC],
                num_idxs=PC_PAD, num_idxs_reg=PERCORE,
                elem_size=F, elem_step=acc_step,
                queue_num=q % NQ, single_packet=False)
            return
        nc.gpsimd.dma_scatter_add(
            out_ap=acc[:, :F],
            in_ap=bass.AP(astrip_t.tensor, base.offset + half * HCOL * F,
                          [base.ap[0], [F, HCOL], [1, F]]),
            idxs_ap=iscb[:, q * SCC + half * (HPOS // 16):
                         q * SCC + (half + 1) * (HPOS // 16)],
            num_idxs=HPOS,
            num_idxs_reg=HPOS if half == 0 else PERCORE - HPOS,
            elem_size=F, elem_step=acc_step,
            queue_num=q % NQ, single_packet=False)

    # call-0 fast path: direct int16 load so gather 0 isn't gated on the
    # broadcast pipeline (its ~9us transfer then covers the convert latency)
    c0 = sum(calls[0][1]) * 8
    idx0_sb = cst_local.tile([128, c0], mybir.dt.int16, tag="idx0")
    nc.sync.dma_start(out=idx0_sb[:], in_=idx0g[:, :c0])

    # broadcast pipelines are emitted lookahead-1: pass q+1's converts land
    # on DVE between pass q's early reduce ops, so they neither stall the
    # next pass's gathers nor push the whole reduce/scatter chain late
    pass_off = [0] * NPASS
    go = 0
    for q in range(NPASS):
        pass_off[q] = go
        go += pass_cols[q]
    idx_tiles = [None] * NPASS

    def emit_idx_pipeline(q):
        ccols = pass_cols[q]
        idxh_sb = ib.tile([32, ccols], mybir.dt.bfloat16, tag="idxh")
        nc.sync.dma_start(out=idxh_sb[:],
                          in_=idxh[:, pass_off[q]:pass_off[q] + ccols])
        idx_sb = cst_local.tile([128, ccols], mybir.dt.int16, tag=f"idx{q}")
        for o in range(0, ccols, BCH):
            w = min(BCH, ccols - o)
            pidx = psp.tile([128, BCH], mybir.dt.float32, space="PSUM",
                            tag="pidx")
            nc.tensor.matmul(pidx[:, :w], lhsT=repb_sb[:],
                             rhs=idxh_sb[:, o:o + w],
                             start=True, stop=True)
            nc.vector.tensor_scalar_add(idx_sb[:, o:o + w],
                                        pidx[:, :w], 0.0)
        idx_tiles[q] = idx_sb

    emit_idx_pipeline(0)

    call_in_pass = 0
    for (q, cs) in calls:
        if q != cur_pass:
            if astrip is not None:
                # high-degree half (cols [0, HCOL)) finalizes at pass end
                flush_half(cur_pass, astrip,
                           0 if split_scatter else 1)
            cur_pass = q
            pass_goff = goff
            call_in_pass = 0
            if idx_tiles[q] is None:
                emit_idx_pipeline(q)
            idx_sb = idx_tiles[q]
            astrip = ap_.tile([128, NCOLS * F], stage_dt, tag="astrip")
        tot = sum(cs)
        ni = tot * 128
        stage = stp.tile([128, STAGE_COLS * F], stage_dt, tag="stage")
        lo = goff - pass_goff
        if q == 0 and lo == 0:
            idx_view = idx0_sb[:, :tot * 8]
        else:
            idx_view = idx_sb[:, lo:lo + tot * 8]
        _dma_gather_thin(
            nc.gpsimd,
            out_ap=bass.AP(stage.tensor, stage[:].offset,
                           [stage[:].ap[0], [F, tot], [1, F]]),
            in_ap=tab_col_of(q),
            idxs_ap=idx_view,
            num_idxs=ni, elem_size=gather_elem, elem_step=gather_step,
            queue_num=qn)
        qn = (qn + 1) % NQ

        # prefix-add tree over the call's k-blocks (c nonincreasing), then
        # one add (or init copy) into astrip[0 : c_first*F)
        if no_reduce:
            goff += tot * 8
            call_in_pass += 1
            continue
        sap0 = stage[:].ap[0]
        soff = stage[:].offset
        blocks = []
        o = 0
        for ck in cs:
            blocks.append((o, ck))
            o += ck
        with nc.allow_low_precision(reason="short partial sums, tree depth"):
            while len(blocks) > 1:
                nxt = []
                for a in range(0, len(blocks) - 1, 2):
                    (o0, c0b), (o1, c1b) = blocks[a], blocks[a + 1]
                    v0 = bass.AP(stage.tensor, soff + o0 * F,
                                 [sap0, [F, c1b], [1, F]])
                    v1 = bass.AP(stage.tensor, soff + o1 * F,
                                 [sap0, [F, c1b], [1, F]])
                    nc.vector.tensor_tensor(out=v0, in0=v0, in1=v1,
                                            op=mybir.AluOpType.add)
                    nxt.append((o0, c0b))
                if len(blocks) % 2:
                    nxt.append(blocks[-1])
                blocks = nxt
            (o0, cfin) = blocks[0]
            srcap = bass.AP(stage.tensor, soff + o0 * F,
                            [sap0, [F, cfin], [1, F]])
            dstap = bass.AP(astrip.tensor, astrip[:].offset,
                            [astrip[:].ap[0], [F, cfin], [1, F]])
            if call_in_pass == 0:
                # c_0 == NCOLS (K >= 1 everywhere): initializes all of astrip
                nc.vector.tensor_scalar_add(out=dstap, in0=srcap, scalar1=0.0)
            else:
                nc.vector.tensor_tensor(out=dstap, in0=dstap, in1=srcap,
                                        op=mybir.AluOpType.add)
        goff += tot * 8
        if split_scatter and call_in_pass == last_big[q]:
            # low-degree half's tiles are never touched by later (smaller-c)
            # calls of this pass
            flush_half(q, astrip, 1)
        if call_in_pass == 1 and q + 1 < NPASS and idx_tiles[q + 1] is None:
            emit_idx_pipeline(q + 1)
        call_in_pass += 1
    flush_half(cur_pass, astrip, 0 if split_scatter else 1)


def _build_agg1(calls, cols_g, skip_epi=False, no_scatter=False,
                no_reduce=False):
    """Layer-1 aggregation + self add + epilogue t2 = relu(...) @ W2."""
    bass, bacc, tile, mybir, _ = _bass_mods()
    from contextlib import ExitStack
    from concourse.masks import make_identity
    bf = mybir.dt.bfloat16
    f32 = mybir.dt.float32
    NQ = 4
    nc = bacc.Bacc(num_swdge_queues=NQ, dynamic_dma_scratch_size=8192 * NQ)
    tabs = [nc.declare_dram_parameter(f"tab{c}", [L1_ROWS, 128], bf,
                                      isOutput=False) for c in range(2)]
    c0 = sum(calls[0][1]) * 8
    idx0g = nc.declare_dram_parameter("idx0g", [128, c0], mybir.dt.int16,
                                      isOutput=False)
    idxh = nc.declare_dram_parameter("idxh", [32, cols_g], bf, isOutput=False)
    repb = nc.declare_dram_parameter("repb", [32, 128], bf, isOutput=False)
    idxsc = nc.declare_dram_parameter("idxsc", [128, NPASS * (PC_PAD // 16)],
                                      mybir.dt.int16, isOutput=False)
    disp = nc.declare_dram_parameter("disp", [128, NCOLS], f32, isOutput=False)
    selfh = nc.declare_dram_parameter("selfh", [128, NCOLS * HID], bf,
                                      isOutput=False)
    W2 = nc.declare_dram_parameter("W2", [HID, OUT], bf, isOutput=False)
    acc = nc.declare_dram_parameter("acc", [PC_PAD, 128], bf, isOutput=True)
    out = nc.declare_dram_parameter("out", [128, NCOLS * OUT], f32,
                                    isOutput=True)

    with tile.TileContext(nc) as tc, ExitStack() as ctx:
        cst = ctx.enter_context(tc.tile_pool(name="cst", bufs=1))
        ep = ctx.enter_context(tc.tile_pool(name="ep", bufs=3))
        ps = ctx.enter_context(tc.tile_pool(name="ps", bufs=4, space="PSUM"))

        dis_sb = cst.tile([128, NCOLS], f32, tag="dis")
        nc.sync.dma_start(out=dis_sb[:], in_=disp[:, :])
        w2t = cst.tile([HID, OUT], bf, tag="w2t")
        nc.sync.dma_start(out=w2t[:], in_=W2[:, :])
        ident = cst.tile([128, 128], bf, tag="ident")
        make_identity(nc, ident[:])

        _common_agg(nc, bass, tile, mybir, ctx, tc, calls, tabs, idx0g, idxh,
                    repb, idxsc,
                    F=HID, stage_dt=bf, gather_elem=HID, gather_step=128,
                    tab_col_of=lambda q: tabs[q // 2][:, (q % 2) * HID:
                                                      (q % 2 + 1) * HID],
                    acc=acc, acc_step=128, NQ=NQ,
                    no_scatter=no_scatter, no_reduce=no_reduce)

        # ---- epilogue (GE-chunked reads of the p-major bf16 accumulator;
        # acc rows are 128-wide with cols HID..128 zero from donation).
        # selfh comes in with b1/dis pre-folded on host, so
        # a1 = relu(dis^2 * (S + selfh)); PSUM work is batched PB columns per
        # ACT copy to amortize the ~370ns scalar-engine access latency. ----
        GE = 10
        PB = 5
        for g0 in ([] if skip_epi else range(0, NCOLS, GE)):
            ng = min(GE, NCOLS - g0)
            sS = ep.tile([128, GE * 128], bf, tag="sS")
            accb = acc[:, :]
            nc.sync.dma_start(
                out=sS[:, :ng * 128].rearrange("p (m f) -> p m f", m=ng),
                in_=bass.AP(accb.tensor, accb.offset + g0 * 128,
                            [[NCOLS * 128, 128], [128, ng], [1, 128]]))
            selft = ep.tile([128, GE * HID], bf, tag="selft")
            nc.sync.dma_start(out=selft[:, :ng * HID],
                              in_=selfh[:, g0 * HID:(g0 + ng) * HID])
            svs = bass.AP(sS.tensor, sS[:].offset,
                          [sS[:].ap[0], [128, ng], [1, HID]])
            selfv = bass.AP(selft.tensor, selft[:].offset,
                            [selft[:].ap[0], [HID, ng], [1, HID]])
            with nc.allow_low_precision(reason="bf16 self add"):
                nc.vector.tensor_tensor(out=svs, in0=svs, in1=selfv,
                                        op=mybir.AluOpType.add)
            a1 = ep.tile([128, GE * HID], bf, tag="a1")
            dview = bass.AP(dis_sb.tensor, dis_sb[:].offset + g0,
                            [dis_sb[:].ap[0], [1, ng], [0, HID]])
            sv = bass.AP(sS.tensor, sS[:].offset,
                         [sS[:].ap[0], [128, ng], [1, HID]])
            av = bass.AP(a1.tensor, a1[:].offset,
                         [a1[:].ap[0], [HID, ng], [1, HID]])
            with nc.allow_low_precision(reason="bf16 epilogue"):
                nc.vector.tensor_tensor(out=av, in0=sv, in1=dview,
                                        op=mybir.AluOpType.mult)
                nc.vector.tensor_scalar_max(a1[:, :ng * HID],
                                            a1[:, :ng * HID], 0.0)
            ostrip = ep.tile([128, GE * OUT], f32, tag="ostrip")
            for h0 in range(0, ng, PB):
                nh = min(PB, ng - h0)
                putb = ps.tile([HID, PB * 128], bf, space="PSUM", tag="putb")
                for j in range(nh):
                    nc.tensor.transpose(
                        out=putb[:, j * 128:(j + 1) * 128],
                        in_=a1[:, (h0 + j) * HID:(h0 + j + 1) * HID],
                        identity=ident[:])
                utb = ep.tile([HID, PB * 128], bf, tag="utb")
                nc.scalar.activation(out=utb[:, :nh * 128],
                                     in_=putb[:, :nh * 128],
                                     func=mybir.ActivationFunctionType.Copy)
                pob = ps.tile([128, PB * OUT], f32, space="PSUM", tag="pob")
                for j in range(nh):
                    nc.tensor.matmul(pob[:, j * OUT:(j + 1) * OUT],
                                     lhsT=utb[:, j * 128:(j + 1) * 128],
                                     rhs=w2t[:], start=True, stop=True)
                nc.scalar.activation(out=ostrip[:, h0 * OUT:(h0 + nh) * OUT],
                                     in_=pob[:, :nh * OUT],
                                     func=mybir.ActivationFunctionType.Copy)
            nc.sync.dma_start(out=out[:, g0 * OUT:(g0 + ng) * OUT],
                              in_=ostrip[:, :ng * OUT])
    nc.compile()
    return nc


def _build_agg2(calls, cols_g, skip_epi=False, no_scatter=False,
                no_reduce=False):
    """Layer-2 aggregation of 2-wide f32 + self add + S2*dis + b2."""
    bass, bacc, tile, mybir, _ = _bass_mods()
    from contextlib import ExitStack
    f32 = mybir.dt.float32
    NQ = 4
    nc = bacc.Bacc(num_swdge_queues=NQ, dynamic_dma_scratch_size=8192 * NQ)
    tabs = [nc.declare_dram_parameter(f"tab{c}", [L2_ROWS, 64], f32,
                                      isOutput=False) for c in range(NPASS)]
    bf = mybir.dt.bfloat16
    c0 = sum(calls[0][1]) * 8
    idx0g = nc.declare_dram_parameter("idx0g", [128, c0], mybir.dt.int16,
                                      isOutput=False)
    idxh = nc.declare_dram_parameter("idxh", [32, cols_g], bf, isOutput=False)
    repb = nc.declare_dram_parameter("repb", [32, 128], bf, isOutput=False)
    idxsc = nc.declare_dram_parameter("idxsc", [128, NPASS * (PC_PAD // 16)],
                                      mybir.dt.int16, isOutput=False)
    disp = nc.declare_dram_parameter("disp", [128, NCOLS], f32, isOutput=False)
    b2b = nc.declare_dram_parameter("b2b", [128, OUT], f32, isOutput=False)
    selft2 = nc.declare_dram_parameter("selft2", [128, NCOLS * OUT], f32,
                                       isOutput=False)
    acc = nc.declare_dram_parameter("acc", [PC_PAD, 64], f32, isOutput=True)
    out = nc.declare_dram_parameter("out", [128, NCOLS * OUT], f32,
                                    isOutput=True)

    with tile.TileContext(nc) as tc, ExitStack() as ctx:
        cst = ctx.enter_context(tc.tile_pool(name="cst", bufs=1))
        big = ctx.enter_context(tc.tile_pool(name="big", bufs=1))

        dis_sb = cst.tile([128, NCOLS], f32, tag="dis")
        nc.sync.dma_start(out=dis_sb[:], in_=disp[:, :])
        b2t = cst.tile([128, OUT], f32, tag="b2t")
        nc.sync.dma_start(out=b2t[:], in_=b2b[:, :])

        _common_agg(nc, bass, tile, mybir, ctx, tc, calls, tabs, idx0g, idxh,
                    repb, idxsc,
                    F=OUT, stage_dt=f32, gather_elem=OUT, gather_step=64,
                    tab_col_of=lambda q: tabs[q][:, :OUT],
                    acc=acc, acc_step=64, NQ=NQ, split_scatter=True,
                    no_scatter=no_scatter, no_reduce=no_reduce,
                    astrip_bufs=4, stage_bufs=8)

        # ---- epilogue: out = (S2 + self)*dis + b2; read only the 2 used
        # f32 of each 64-wide acc row (8B strided elems ride the 7ns floor)
        if skip_epi:
            nc.compile()
            return nc
        sS = big.tile([128, NCOLS * OUT], f32, tag="sS")
        accb = acc[:, :]
        nc.sync.dma_start(
            out=sS[:].rearrange("p (m f) -> p m f", m=NCOLS),
            in_=bass.AP(accb.tensor, accb.offset,
                        [[NCOLS * 64, 128], [64, NCOLS], [1, OUT]]))
        selft = big.tile([128, NCOLS * OUT], f32, tag="selft")
        nc.sync.dma_start(out=selft[:], in_=selft2[:, :])
        nc.vector.tensor_tensor(
            out=sS[:].rearrange("p (m f) -> p m f", m=NCOLS),
            in0=sS[:].rearrange("p (m f) -> p m f", m=NCOLS),
            in1=selft[:].rearrange("p (m f) -> p m f", m=NCOLS),
            op=mybir.AluOpType.add)
        dview = bass.AP(dis_sb.tensor, dis_sb[:].offset,
                        [dis_sb[:].ap[0], [1, NCOLS], [0, OUT]])
        sv = bass.AP(sS.tensor, sS[:].offset,
                     [sS[:].ap[0], [OUT, NCOLS], [1, OUT]])
        b2view = bass.AP(b2t.tensor, b2t[:].offset,
                         [b2t[:].ap[0], [0, NCOLS], [1, OUT]])
        nc.vector.tensor_tensor(out=sv, in0=sv, in1=dview,
                                op=mybir.AluOpType.mult)
        nc.vector.tensor_tensor(out=sv, in0=sv, in1=b2view,
                                op=mybir.AluOpType.add)
        nc.sync.dma_start(out=out[:, :], in_=sS[:])
    nc.compile()
    return nc


def _pmajor(arr_pad):
    """[PC_PAD, F] node order -> [128, NCOLS*F] p-major."""
    F = arr_pad.shape[1]
    return np.ascontiguousarray(
        arr_pad.reshape(NCOLS, 128, F).transpose(1, 0, 2).reshape(128, NCOLS * F))


def _unpmajor(arr_pm, F):
    """[128, NCOLS*F] p-major -> [PC_PAD, F] node order."""
    return np.ascontiguousarray(
        arr_pm.reshape(128, NCOLS, F).transpose(1, 0, 2).reshape(PC_PAD, F))


def kernel(x, edge_index, W1, b1, W2, b2):
    import ml_dtypes
    bf16 = ml_dtypes.bfloat16
    x = np.asarray(x, dtype=np.float32)
    W1 = np.asarray(W1, dtype=np.float32)
    b1 = np.asarray(b1, dtype=np.float32)
    W2 = np.asarray(W2, dtype=np.float32)
    b2 = np.asarray(b2, dtype=np.float32)

    bass, bacc, tile, mybir, run_spmd = _bass_mods()

    dis, (c1, x1, s1, nd1), (c2, x2, s2, nd2) = _host_prep(edge_index)
    cores = list(range(NC))

    # idx broadcast operands: hi/lo bf16 rows of the 16-partition wrap, and
    # the stacked replication matrix (256*rep | rep)
    def _idx_ops(xg, calls):
        base = xg[:, :16, :].astype(np.int32)   # [NC, 16, cols]
        idxh = np.concatenate([base // 256, base % 256], axis=1).astype(bf16)
        c0 = sum(calls[0][1]) * 8
        idx0g = np.ascontiguousarray(xg[:, :, :c0])
        return idxh, idx0g

    repb = np.zeros((32, 128), bf16)
    for p in range(128):
        repb[p % 16, p] = 256.0
        repb[16 + p % 16, p] = 1.0

    def _dpad(i):
        dp = np.concatenate([dis[i * PERCORE:(i + 1) * PERCORE],
                             np.ones(PC_PAD - PERCORE, np.float32)])
        return dp

    disps = [np.ascontiguousarray(_dpad(i).reshape(NCOLS, 128).T)
             for i in cores]

    # ---- launch 1: mm ----
    nc1 = _build_mm()
    in1 = []
    for i in cores:
        xT = np.zeros((E_CH, PC_PAD), bf16)
        xT[:, :PERCORE] = x[i * PERCORE:(i + 1) * PERCORE].T.astype(bf16)
        in1.append({"xT": xT, "W1": W1.astype(bf16), "disp": disps[i]})
    r1 = run_spmd(nc1, in1, core_ids=cores)
    h1p = np.concatenate([
        _unpmajor(np.asarray(r1.results[i]["out"]), HID)[:PERCORE]
        for i in cores])  # [N, HID] bf16

    # ---- host: pack layer-1 pair tables ----
    tabs1 = []
    for c in range(2):
        t = np.zeros((L1_ROWS, 128), bf16)
        t[:25000] = h1p[c * 50000:(c + 1) * 50000].reshape(25000, 128)
        tabs1.append(t)

    # ---- launch 2 ----
    nc2 = _build_agg1(c1, x1.shape[2])
    idxh1, idx0g1 = _idx_ops(x1, c1)
    in2 = []
    for i in cores:
        dp = np.concatenate([dis[i * PERCORE:(i + 1) * PERCORE],
                             np.ones(PC_PAD - PERCORE, np.float32)])
        # fold the bias in: a1 = relu(dis^2*(S + selfh + b1/dis)) on device
        h1pad = np.zeros((PC_PAD, HID), np.float32)
        h1pad[:PERCORE] = h1p[i * PERCORE:(i + 1) * PERCORE].astype(np.float32)
        h1pad += b1[None, :] / dp[:, None]
        m = {f"tab{c}": tabs1[c] for c in range(2)}
        m.update({
            "idxh": idxh1[i],
            "idx0g": idx0g1[i],
            "repb": repb,
            "idxsc": np.concatenate([s1[i, q] for q in range(NPASS)], axis=1),
            # epilogue constant: dis^2 per destination
            "disp": np.ascontiguousarray((dp * dp).reshape(NCOLS, 128).T),
            "selfh": _pmajor(h1pad.astype(bf16)),
            "W2": W2.astype(bf16),
        })
        in2.append(m)
    r2 = run_spmd(nc2, in2, core_ids=cores)
    t2 = np.concatenate([
        _unpmajor(np.asarray(r2.results[i]["out"]), OUT)[:PERCORE]
        for i in cores])  # [N, 2] f32

    # ---- host: pack layer-2 tables ----
    tabs2 = []
    for c in range(NPASS):
        t = np.zeros((L2_ROWS, 64), np.float32)
        t[:25000, :OUT] = t2[c * 25000:(c + 1) * 25000]
        tabs2.append(t)

    # ---- launch 3 ----
    nc3 = _build_agg2(c2, x2.shape[2])
    idxh2, idx0g2 = _idx_ops(x2, c2)
    b2bc = np.broadcast_to(b2, (128, OUT)).astype(np.float32).copy()
    in3 = []
    for i in cores:
        t2pad = np.zeros((PC_PAD, OUT), np.float32)
        t2pad[:PERCORE] = t2[i * PERCORE:(i + 1) * PERCORE]
        m = {f"tab{c}": tabs2[c] for c in range(NPASS)}
        m.update({
            "idxh": idxh2[i],
            "idx0g": idx0g2[i],
            "repb": repb,
            "idxsc": np.concatenate([s2[i, q] for q in range(NPASS)], axis=1),
            "disp": disps[i],
            "b2b": b2bc,
            "selft2": _pmajor(t2pad),
        })
        in3.append(m)
    r3 = run_spmd(nc3, in3, core_ids=cores)
    outv = np.concatenate([
        _unpmajor(np.asarray(r3.results[i]["out"]), OUT)[:PERCORE]
        for i in cores])
    return outv.astype(np.float32)



# revision 75
# speedup vs baseline: 1.0154x; 1.0154x over previous
"""Two-layer GCN on 8 Trainium2 NeuronCores — v2.1 (descriptor-lean).

HW is SWDGE-descriptor-rate bound (~4-5ns/desc at 4 queues, size-independent),
so v2.1 minimizes descriptor COUNT beyond the one-desc-per-edge gather:
- Self-loops never gathered: own-shard h1p/t2 terms are bulk-loaded p-major
  (~128 descs) and added on DVE.
- Per-pass partials scatter-add (f32, dense 256B rows) into a p-major
  accumulator keyed by true destination, so the epilogue re-reads the whole
  accumulator with 128 descriptors (one contiguous run per partition).
- All launch inputs/outputs that are per-destination use the plain p-major
  [128, 98*F] layout (dst = n*128 + p at [p, n*F:(n+1)*F]) — bulk DMA.
Layer tables as in v2: layer-1 bf16 pair-packed rows (128B descs via the
256B-stride sub-row gather), layer-2 f32 2-wide (8B descs). Gather perms are
per-pass global degree sorts (tight ~5% slot padding).
"""
import numpy as np

N = 100000
E_CH = 128
HID = 64
OUT = 2
NC = 8
PERCORE = 12500
PC_PAD = 12544
NCOLS = 98               # p-major columns (dst = n*128+p, n in [0,98))
NPASS = 4
L1_ROWS = 25001
L2_ROWS = 25001
ZROW = 25000
MAXPOS = 16384           # gather positions per call (slot-major k-blocks);
                         # stage tile is [128, (MAXPOS//128)*F]
STAGE_COLS = MAXPOS // 128


def _wrap_idx(vals):
    ni = len(vals)
    assert ni % 16 == 0
    return np.tile(vals.reshape(ni // 16, 16).T, (8, 1))


def _prep_layer(src, dst, pass_of, row_of):
    """Per-layer prep: per-pass degree-sorted perms, per-tile shared-K slot
    matrices, slot-major (k-block) gather calls, gather idx, and p-major
    scatter idx. No self-loops here.

    Slot-major: positions are ordered (k, tile, p) with c_k = #tiles whose
    K exceeds k. Each call covers consecutive k's with sum(c_k)*128 <=
    MAXPOS, so slot padding is per-tile-max only (~3%) and the reduce is a
    prefix-add tree on DVE (packed bf16/f32, 2x mode) instead of a strided
    tensor_reduce."""
    p = pass_of(src)
    r = row_of(src)
    core = dst // PERCORE
    dloc = dst % PERCORE

    cnt = np.bincount(dst * NPASS + p, minlength=N * NPASS).reshape(N, NPASS)

    perms = np.zeros((NC, NPASS, PC_PAD), np.int64)
    K = np.zeros((NPASS, PC_PAD // 128), np.int64)
    for i in range(NC):
        c0 = cnt[i * PERCORE:(i + 1) * PERCORE]
        for q in range(NPASS):
            pp = np.argsort(-c0[:, q], kind="stable")
            pp = np.concatenate([pp, np.arange(PERCORE, PC_PAD)])
            perms[i, q] = pp
            cc = np.concatenate([c0[:, q], np.zeros(PC_PAD - PERCORE, np.int64)])
            kt = cc[pp].reshape(-1, 128).max(axis=1)
            K[q] = np.maximum(K[q], kt)
    K = np.maximum(K, 1)

    invperms = np.zeros((NC, NPASS, PC_PAD), np.int64)
    for i in range(NC):
        for q in range(NPASS):
            invperms[i, q][perms[i, q]] = np.arange(PC_PAD)
    Ms = [[None] * NPASS for _ in range(NC)]
    for i in range(NC):
        ec = core == i
        for q in range(NPASS):
            sel = ec & (p == q)
            ed = dloc[sel]
            es = r[sel]
            order = np.argsort(ed, kind="stable")
            ed = ed[order]
            es = es[order]
            starts = np.searchsorted(ed, np.arange(PERCORE))
            rank = np.arange(len(ed)) - starts[ed]
            kmax = int(K[q].max())
            M = np.full((PC_PAD, kmax), ZROW, np.int16)
            M[invperms[i, q][ed], rank] = es.astype(np.int16)
            Ms[i][q] = M

    # slot-major call packing: per pass, greedy k-blocks under MAXPOS
    calls = []  # (pass, [c_k, ...], k0, tile0)
    for q in range(NPASS):
        kmax = int(K[q].max())
        cs_all = [int((K[q] > k).sum()) for k in range(kmax)]
        cur = []
        k0 = 0
        for ck in cs_all:
            if cur and (sum(cur) + ck) * 128 > MAXPOS:
                calls.append((q, cur, k0, 0))
                k0 += len(cur)
                cur = []
            cur.append(ck)
        if cur:
            calls.append((q, cur, k0, 0))
    # halve the very first call: its descgen (~5.3us) gates the launch ramp
    q0, cs0, _, _ = calls[0]
    if len(cs0) == 1 and cs0[0] > 1:
        a = cs0[0] // 2
        calls[0:1] = [(q0, [a], 0, 0), (q0, [cs0[0] - a], 0, a)]

    idxg = []
    for i in range(NC):
        parts = []
        for (q, cs, k0, t0) in calls:
            vals = np.concatenate(
                [Ms[i][q][t0 * 128:(t0 + ck) * 128, k0 + j].reshape(ck, 128)
                 for j, ck in enumerate(cs)], axis=0)
            parts.append(_wrap_idx(vals.ravel()))
        idxg.append(np.concatenate(parts, axis=1))
    idxg = np.stack(idxg)  # [NC, 128, COLS_G]

    # scatter idx per (core, pass): perm position i -> p-major acc row of the
    # true dst; pad positions (perm rank >= PERCORE, trailing) -> -1
    idxsc = np.zeros((NC, NPASS, 128, PC_PAD // 16), np.int16)
    for i in range(NC):
        for q in range(NPASS):
            d = perms[i, q]
            v = ((d % 128) * NCOLS + d // 128).astype(np.int16)
            v[PERCORE:] = -1
            idxsc[i, q] = _wrap_idx(v)

    ndesc = int(K.sum()) * 128
    return calls, idxg, idxsc, ndesc


def _host_prep(edge_index):
    src = np.asarray(edge_index[0], dtype=np.int64)
    dst = np.asarray(edge_index[1], dtype=np.int64)
    deg = np.bincount(dst, minlength=N).astype(np.float64) + 1.0
    dis = (1.0 / np.sqrt(deg)).astype(np.float32)

    # no appended self-loops; added directly in the epilogues
    l1 = _prep_layer(src, dst,
                     lambda s: (s // 50000) * 2 + (s % 2),
                     lambda s: (s % 50000) // 2)
    l2 = _prep_layer(src, dst,
                     lambda s: s // 25000,
                     lambda s: s % 25000)
    return dis, l1, l2


def _bass_mods():
    import sys
    if "/opt/trn_rl_repo" not in sys.path:
        sys.path.insert(0, "/opt/trn_rl_repo")
    import concourse.bass as bass
    import concourse.bacc as bacc
    import concourse.tile as tile
    from concourse import mybir
    from concourse.bass_utils import run_bass_kernel_spmd
    return bass, bacc, tile, mybir, run_bass_kernel_spmd


def _dma_gather_thin(gp, out_ap, in_ap, idxs_ap, num_idxs, elem_size,
                     elem_step, queue_num):
    from concourse import mybir
    gp._assert_queue_num(queue_num)
    assert idxs_ap.dtype == mybir.dt.int16
    stride_bytes = elem_step * mybir.dt.size(in_ap.dtype)
    assert stride_bytes % 256 == 0 and stride_bytes // 256 < 256
    assert in_ap.ap[-1][1] == elem_size
    assert in_ap.ap[0][0] == elem_step
    _in_ap = gp.lower_ap_dma(in_ap, for_custom_bir_dma=True)
    _idxs_ap = gp.lower_ap(idxs_ap)
    _out_ap = gp.lower_ap(out_ap)
    return gp.add_instruction(
        mybir.InstDMAGatherAnt(
            name=gp.bass.get_next_instruction_name(),
            ins=[*_in_ap, _idxs_ap, gp.lower_val_access(gp.to_reg(num_idxs))],
            outs=[_out_ap],
            transpose=False,
            num_idxs=num_idxs,
            elem_size=elem_size,
            stride_bytes_256=stride_bytes // 256,
            gen_mode=0,
            single_packet=False,
            queue_num=queue_num,
            sbuf_tokens_per_rank=0,
            sbuf_free_dim_per_rank=0,
            sbuf_free_dim_pad_per_rank=0,
            sbuf_byte_offset=0,
        )
    )


def _build_mm():
    """h1p = (x @ W1) * dis for own shard, bf16, p-major output."""
    bass, bacc, tile, mybir, _ = _bass_mods()
    from contextlib import ExitStack
    nc = bacc.Bacc()
    bf = mybir.dt.bfloat16
    xT = nc.declare_dram_parameter("xT", [E_CH, PC_PAD], bf, isOutput=False)
    W1 = nc.declare_dram_parameter("W1", [E_CH, HID], bf, isOutput=False)
    disp = nc.declare_dram_parameter("disp", [128, NCOLS], mybir.dt.float32,
                                     isOutput=False)
    out = nc.declare_dram_parameter("out", [128, NCOLS * HID], bf,
                                    isOutput=True)
    G = 14
    with tile.TileContext(nc) as tc, ExitStack() as ctx:
        wp = ctx.enter_context(tc.tile_pool(name="wp", bufs=1))
        sb = ctx.enter_context(tc.tile_pool(name="sb", bufs=3))
        ps = ctx.enter_context(tc.tile_pool(name="ps", bufs=4, space="PSUM"))
        w1 = wp.tile([E_CH, HID], bf, tag="w1")
        nc.sync.dma_start(out=w1[:], in_=W1[:, :])
        dis_sb = wp.tile([128, NCOLS], mybir.dt.float32, tag="dis")
        nc.sync.dma_start(out=dis_sb[:], in_=disp[:, :])
        PB = 7
        for g in range(0, NCOLS, G):
            ng = min(G, NCOLS - g)
            xt = sb.tile([E_CH, G * 128], bf, tag="xt")
            nc.sync.dma_start(out=xt[:, :ng * 128],
                              in_=xT[:, g * 128:(g + ng) * 128])
            ot = sb.tile([128, G * HID], bf, tag="ot")
            for h0 in range(0, ng, PB):
                nh = min(PB, ng - h0)
                pt = ps.tile([128, PB * HID], mybir.dt.float32, space="PSUM",
                             tag="pt")
                for j in range(nh):
                    nc.tensor.matmul(pt[:, j * HID:(j + 1) * HID],
                                     lhsT=xt[:, (h0 + j) * 128:
                                             (h0 + j + 1) * 128],
                                     rhs=w1[:], start=True, stop=True)
                # scale each column's HID block by its dis in one strided op
                dview = bass.AP(dis_sb.tensor, dis_sb[:].offset + g + h0,
                                [dis_sb[:].ap[0], [1, nh], [0, HID]])
                pv = bass.AP(pt.tensor, pt[:].offset,
                             [pt[:].ap[0], [HID, nh], [1, HID]])
                ov = bass.AP(ot.tensor, ot[:].offset + h0 * HID,
                             [ot[:].ap[0], [HID, nh], [1, HID]])
                with nc.allow_low_precision(reason="bf16 h1 staging"):
                    nc.vector.tensor_tensor(out=ov, in0=pv, in1=dview,
                                            op=mybir.AluOpType.mult)
            nc.sync.dma_start(out=out[:, g * HID:(g + ng) * HID],
                              in_=ot[:, :ng * HID])
    nc.compile()
    return nc


def _common_agg(nc, bass, tile, mybir, ctx, tc, calls, tabs, idx0g, idxh,
                repb, idxsc, F, stage_dt, gather_elem, gather_step,
                tab_col_of, acc, acc_step, NQ, split_scatter=True,
                no_scatter=False, no_reduce=False, astrip_bufs=2,
                stage_bufs=4):
    """Shared gather/reduce/scatter pipeline. astrip is compact
    [128, NCOLS*F] in stage_dt; scatter writes F elems per destination into
    `acc` (row stride acc_step elems = 256B; untouched columns stay zero via
    output zero-donation).

    Gather indices arrive as hi/lo bf16 [32, cols] (idxh) and are broadcast
    to the q7-required 8x-replicated int16 [128, cols] layout on-chip:
    PE matmul against repb (256*rep | rep) then an exact f32->int16 convert
    on DVE. This cuts idx HBM traffic 4x. Call 0 uses a small direct int16
    load (idx0g) so the first gather isn't gated on the broadcast pipeline."""
    ib = ctx.enter_context(tc.tile_pool(name="ib", bufs=2))
    stp = ctx.enter_context(tc.tile_pool(name="stp", bufs=stage_bufs))
    ap_ = ctx.enter_context(tc.tile_pool(name="ap", bufs=astrip_bufs))
    psp = ctx.enter_context(tc.tile_pool(name="psp", bufs=2, space="PSUM"))

    cst_local = ctx.enter_context(tc.tile_pool(name="cstl", bufs=1))
    SCC = PC_PAD // 16
    iscb = cst_local.tile([128, NPASS * SCC], mybir.dt.int16, tag="iscb")
    iscb_loaded = [False]
    repb_sb = cst_local.tile([32, 128], mybir.dt.bfloat16, tag="repb")
    nc.sync.dma_start(out=repb_sb[:], in_=repb[:, :])
    BCH = 512  # psum-chunk columns per broadcast matmul

    def ensure_iscb():
        # deferred so the launch ramp isn't spent on scatter indices
        if not iscb_loaded[0]:
            nc.sync.dma_start(out=iscb[:], in_=idxsc[:, :])
            iscb_loaded[0] = True

    qn = 0
    goff = 0
    cur_pass = -1
    idx_sb = None
    idx0_sb = None
    pass_goff = 0
    astrip = None
    pass_cols = {}
    pass_ncalls = {}
    for (q, cs, _k0, _t0) in calls:
        pass_cols[q] = pass_cols.get(q, 0) + sum(cs) * 8
        pass_ncalls[q] = pass_ncalls.get(q, 0) + 1

    HCOL = NCOLS // 2          # 49 astrip columns per scatter half
    HPOS = HCOL * 128          # 6272 positions per half

    # per pass: index (within the pass) of the last call touching any tile
    # >= HCOL; after it, astrip cols [HCOL, NCOLS) are final (c_k shrinks)
    last_big = {}
    seen = {}
    for (q, cs, _k0, t0) in calls:
        j = seen.get(q, 0)
        if t0 + cs[0] > HCOL:
            last_big[q] = j
        seen[q] = j + 1

    def flush_half(q, astrip_t, half):
        if no_scatter:
            return
        ensure_iscb()
        base = astrip_t[:]
        if not split_scatter and half == 1:
            nc.gpsimd.dma_scatter_add(
                out_ap=acc[:, :F],
                in_ap=astrip_t[:].rearrange("p (k f) -> p k f", k=NCOLS),
                idxs_ap=iscb[:, q * SCC:(q + 1) * SCC],
                num_idxs=PC_PAD, num_idxs_reg=PERCORE,
                elem_size=F, elem_step=acc_step,
                queue_num=q % NQ, single_packet=False)
            return
        nc.gpsimd.dma_scatter_add(
            out_ap=acc[:, :F],
            in_ap=bass.AP(astrip_t.tensor, base.offset + half * HCOL * F,
                          [base.ap[0], [F, HCOL], [1, F]]),
            idxs_ap=iscb[:, q * SCC + half * (HPOS // 16):
                         q * SCC + (half + 1) * (HPOS // 16)],
            num_idxs=HPOS,
            num_idxs_reg=HPOS if half == 0 else PERCORE - HPOS,
            elem_size=F, elem_step=acc_step,
            queue_num=q % NQ, single_packet=False)

    # call-0 fast path: direct int16 load so gather 0 isn't gated on the
    # broadcast pipeline (its ~9us transfer then covers the convert latency)
    c0 = sum(calls[0][1]) * 8
    idx0_sb = cst_local.tile([128, c0], mybir.dt.int16, tag="idx0")
    nc.sync.dma_start(out=idx0_sb[:], in_=idx0g[:, :c0])

    # broadcast pipelines are emitted lookahead-1: pass q+1's converts land
    # on DVE between pass q's early reduce ops, so they neither stall the
    # next pass's gathers nor push the whole reduce/scatter chain late
    pass_off = [0] * NPASS
    go = 0
    for q in range(NPASS):
        pass_off[q] = go
        go += pass_cols[q]
    idx_tiles = [None] * NPASS

    def emit_idx_pipeline(q):
        ccols = pass_cols[q]
        idxh_sb = ib.tile([32, ccols], mybir.dt.bfloat16, tag="idxh")
        nc.sync.dma_start(out=idxh_sb[:],
                          in_=idxh[:, pass_off[q]:pass_off[q] + ccols])
        idx_sb = cst_local.tile([128, ccols], mybir.dt.int16, tag=f"idx{q}")
        for o in range(0, ccols, BCH):
            w = min(BCH, ccols - o)
            pidx = psp.tile([128, BCH], mybir.dt.float32, space="PSUM",
                            tag="pidx")
            nc.tensor.matmul(pidx[:, :w], lhsT=repb_sb[:],
                             rhs=idxh_sb[:, o:o + w],
                             start=True, stop=True)
            nc.vector.tensor_scalar_add(idx_sb[:, o:o + w],
                                        pidx[:, :w], 0.0)
        idx_tiles[q] = idx_sb

    emit_idx_pipeline(0)

    call_in_pass = 0
    for (q, cs, _k0, t0) in calls:
        if q != cur_pass:
            if astrip is not None:
                # high-degree half (cols [0, HCOL)) finalizes at pass end
                flush_half(cur_pass, astrip,
                           0 if split_scatter else 1)
            cur_pass = q
            pass_goff = goff
            call_in_pass = 0
            if idx_tiles[q] is None:
                emit_idx_pipeline(q)
            idx_sb = idx_tiles[q]
            astrip = ap_.tile([128, NCOLS * F], stage_dt, tag="astrip")
        tot = sum(cs)
        ni = tot * 128
        stage = stp.tile([128, STAGE_COLS * F], stage_dt, tag="stage")
        lo = goff - pass_goff
        if q == 0 and lo == 0:
            idx_view = idx0_sb[:, :tot * 8]
        else:
            idx_view = idx_sb[:, lo:lo + tot * 8]
        _dma_gather_thin(
            nc.gpsimd,
            out_ap=bass.AP(stage.tensor, stage[:].offset,
                           [stage[:].ap[0], [F, tot], [1, F]]),
            in_ap=tab_col_of(q),
            idxs_ap=idx_view,
            num_idxs=ni, elem_size=gather_elem, elem_step=gather_step,
            queue_num=qn)
        qn = (qn + 1) % NQ

        # prefix-add tree over the call's k-blocks (c nonincreasing), then
        # one add (or init copy) into astrip[0 : c_first*F)
        if no_reduce:
            goff += tot * 8
            call_in_pass += 1
            continue
        sap0 = stage[:].ap[0]
        soff = stage[:].offset
        blocks = []
        o = 0
        for ck in cs:
            blocks.append((o, ck))
            o += ck
        with nc.allow_low_precision(reason="short partial sums, tree depth"):
            while len(blocks) > 1:
                nxt = []
                for a in range(0, len(blocks) - 1, 2):
                    (o0, c0b), (o1, c1b) = blocks[a], blocks[a + 1]
                    v0 = bass.AP(stage.tensor, soff + o0 * F,
                                 [sap0, [F, c1b], [1, F]])
                    v1 = bass.AP(stage.tensor, soff + o1 * F,
                                 [sap0, [F, c1b], [1, F]])
                    nc.vector.tensor_tensor(out=v0, in0=v0, in1=v1,
                                            op=mybir.AluOpType.add)
                    nxt.append((o0, c0b))
                if len(blocks) % 2:
                    nxt.append(blocks[-1])
                blocks = nxt
            (o0, cfin) = blocks[0]
            srcap = bass.AP(stage.tensor, soff + o0 * F,
                            [sap0, [F, cfin], [1, F]])
            dstap = bass.AP(astrip.tensor, astrip[:].offset + t0 * F,
                            [astrip[:].ap[0], [F, cfin], [1, F]])
            if call_in_pass == 0 or t0 > 0:
                # c_0 == NCOLS (K >= 1 everywhere): initializes all of astrip
                nc.vector.tensor_scalar_add(out=dstap, in0=srcap, scalar1=0.0)
            else:
                nc.vector.tensor_tensor(out=dstap, in0=dstap, in1=srcap,
                                        op=mybir.AluOpType.add)
        goff += tot * 8
        if split_scatter and call_in_pass == last_big[q]:
            # low-degree half's tiles are never touched by later (smaller-c)
            # calls of this pass
            flush_half(q, astrip, 1)
        if call_in_pass == 1 and q + 1 < NPASS and idx_tiles[q + 1] is None:
            emit_idx_pipeline(q + 1)
        call_in_pass += 1
    flush_half(cur_pass, astrip, 0 if split_scatter else 1)


def _build_agg1(calls, cols_g, skip_epi=False, no_scatter=False,
                no_reduce=False):
    """Layer-1 aggregation + self add + epilogue t2 = relu(...) @ W2."""
    bass, bacc, tile, mybir, _ = _bass_mods()
    from contextlib import ExitStack
    from concourse.masks import make_identity
    bf = mybir.dt.bfloat16
    f32 = mybir.dt.float32
    NQ = 4
    nc = bacc.Bacc(num_swdge_queues=NQ, dynamic_dma_scratch_size=8192 * NQ)
    tabs = [nc.declare_dram_parameter(f"tab{c}", [L1_ROWS, 128], bf,
                                      isOutput=False) for c in range(2)]
    c0 = sum(calls[0][1]) * 8
    idx0g = nc.declare_dram_parameter("idx0g", [128, c0], mybir.dt.int16,
                                      isOutput=False)
    idxh = nc.declare_dram_parameter("idxh", [32, cols_g], bf, isOutput=False)
    repb = nc.declare_dram_parameter("repb", [32, 128], bf, isOutput=False)
    idxsc = nc.declare_dram_parameter("idxsc", [128, NPASS * (PC_PAD // 16)],
                                      mybir.dt.int16, isOutput=False)
    disp = nc.declare_dram_parameter("disp", [128, NCOLS], f32, isOutput=False)
    selfh = nc.declare_dram_parameter("selfh", [128, NCOLS * HID], bf,
                                      isOutput=False)
    W2 = nc.declare_dram_parameter("W2", [HID, OUT], bf, isOutput=False)
    acc = nc.declare_dram_parameter("acc", [PC_PAD, 128], bf, isOutput=True)
    out = nc.declare_dram_parameter("out", [128, NCOLS * OUT], f32,
                                    isOutput=True)

    with tile.TileContext(nc) as tc, ExitStack() as ctx:
        cst = ctx.enter_context(tc.tile_pool(name="cst", bufs=1))
        ep = ctx.enter_context(tc.tile_pool(name="ep", bufs=3))
        ps = ctx.enter_context(tc.tile_pool(name="ps", bufs=4, space="PSUM"))

        dis_sb = cst.tile([128, NCOLS], f32, tag="dis")
        nc.sync.dma_start(out=dis_sb[:], in_=disp[:, :])
        w2t = cst.tile([HID, OUT], bf, tag="w2t")
        nc.sync.dma_start(out=w2t[:], in_=W2[:, :])
        ident = cst.tile([128, 128], bf, tag="ident")
        make_identity(nc, ident[:])

        _common_agg(nc, bass, tile, mybir, ctx, tc, calls, tabs, idx0g, idxh,
                    repb, idxsc,
                    F=HID, stage_dt=bf, gather_elem=HID, gather_step=128,
                    tab_col_of=lambda q: tabs[q // 2][:, (q % 2) * HID:
                                                      (q % 2 + 1) * HID],
                    acc=acc, acc_step=128, NQ=NQ,
                    no_scatter=no_scatter, no_reduce=no_reduce)

        # ---- epilogue (GE-chunked reads of the p-major bf16 accumulator;
        # acc rows are 128-wide with cols HID..128 zero from donation).
        # selfh comes in with b1/dis pre-folded on host, so
        # a1 = relu(dis^2 * (S + selfh)); PSUM work is batched PB columns per
        # ACT copy to amortize the ~370ns scalar-engine access latency. ----
        GE = 10
        PB = 5
        for g0 in ([] if skip_epi else range(0, NCOLS, GE)):
            ng = min(GE, NCOLS - g0)
            sS = ep.tile([128, GE * 128], bf, tag="sS")
            accb = acc[:, :]
            nc.sync.dma_start(
                out=sS[:, :ng * 128].rearrange("p (m f) -> p m f", m=ng),
                in_=bass.AP(accb.tensor, accb.offset + g0 * 128,
                            [[NCOLS * 128, 128], [128, ng], [1, 128]]))
            selft = ep.tile([128, GE * HID], bf, tag="selft")
            nc.sync.dma_start(out=selft[:, :ng * HID],
                              in_=selfh[:, g0 * HID:(g0 + ng) * HID])
            svs = bass.AP(sS.tensor, sS[:].offset,
                          [sS[:].ap[0], [128, ng], [1, HID]])
            selfv = bass.AP(selft.tensor, selft[:].offset,
                            [selft[:].ap[0], [HID, ng], [1, HID]])
            with nc.allow_low_precision(reason="bf16 self add"):
                nc.vector.tensor_tensor(out=svs, in0=svs, in1=selfv,
                                        op=mybir.AluOpType.add)
            a1 = ep.tile([128, GE * HID], bf, tag="a1")
            dview = bass.AP(dis_sb.tensor, dis_sb[:].offset + g0,
                            [dis_sb[:].ap[0], [1, ng], [0, HID]])
            sv = bass.AP(sS.tensor, sS[:].offset,
                         [sS[:].ap[0], [128, ng], [1, HID]])
            av = bass.AP(a1.tensor, a1[:].offset,
                         [a1[:].ap[0], [HID, ng], [1, HID]])
            with nc.allow_low_precision(reason="bf16 epilogue"):
                nc.vector.tensor_tensor(out=av, in0=sv, in1=dview,
                                        op=mybir.AluOpType.mult)
                nc.vector.tensor_scalar_max(a1[:, :ng * HID],
                                            a1[:, :ng * HID], 0.0)
            ostrip = ep.tile([128, GE * OUT], f32, tag="ostrip")
            for h0 in range(0, ng, PB):
                nh = min(PB, ng - h0)
                putb = ps.tile([HID, PB * 128], bf, space="PSUM", tag="putb")
                for j in range(nh):
                    nc.tensor.transpose(
                        out=putb[:, j * 128:(j + 1) * 128],
                        in_=a1[:, (h0 + j) * HID:(h0 + j + 1) * HID],
                        identity=ident[:])
                utb = ep.tile([HID, PB * 128], bf, tag="utb")
                nc.scalar.activation(out=utb[:, :nh * 128],
                                     in_=putb[:, :nh * 128],
                                     func=mybir.ActivationFunctionType.Copy)
                pob = ps.tile([128, PB * OUT], f32, space="PSUM", tag="pob")
                for j in range(nh):
                    nc.tensor.matmul(pob[:, j * OUT:(j + 1) * OUT],
                                     lhsT=utb[:, j * 128:(j + 1) * 128],
                                     rhs=w2t[:], start=True, stop=True)
                nc.scalar.activation(out=ostrip[:, h0 * OUT:(h0 + nh) * OUT],
                                     in_=pob[:, :nh * OUT],
                                     func=mybir.ActivationFunctionType.Copy)
            nc.sync.dma_start(out=out[:, g0 * OUT:(g0 + ng) * OUT],
                              in_=ostrip[:, :ng * OUT])
    nc.compile()
    return nc


def _build_agg2(calls, cols_g, skip_epi=False, no_scatter=False,
                no_reduce=False):
    """Layer-2 aggregation of 2-wide f32 + self add + S2*dis + b2."""
    bass, bacc, tile, mybir, _ = _bass_mods()
    from contextlib import ExitStack
    f32 = mybir.dt.float32
    NQ = 4
    nc = bacc.Bacc(num_swdge_queues=NQ, dynamic_dma_scratch_size=8192 * NQ)
    tabs = [nc.declare_dram_parameter(f"tab{c}", [L2_ROWS, 64], f32,
                                      isOutput=False) for c in range(NPASS)]
    bf = mybir.dt.bfloat16
    c0 = sum(calls[0][1]) * 8
    idx0g = nc.declare_dram_parameter("idx0g", [128, c0], mybir.dt.int16,
                                      isOutput=False)
    idxh = nc.declare_dram_parameter("idxh", [32, cols_g], bf, isOutput=False)
    repb = nc.declare_dram_parameter("repb", [32, 128], bf, isOutput=False)
    idxsc = nc.declare_dram_parameter("idxsc", [128, NPASS * (PC_PAD // 16)],
                                      mybir.dt.int16, isOutput=False)
    disp = nc.declare_dram_parameter("disp", [128, NCOLS], f32, isOutput=False)
    b2b = nc.declare_dram_parameter("b2b", [128, OUT], f32, isOutput=False)
    selft2 = nc.declare_dram_parameter("selft2", [128, NCOLS * OUT], f32,
                                       isOutput=False)
    acc = nc.declare_dram_parameter("acc", [PC_PAD, 64], f32, isOutput=True)
    out = nc.declare_dram_parameter("out", [128, NCOLS * OUT], f32,
                                    isOutput=True)

    with tile.TileContext(nc) as tc, ExitStack() as ctx:
        cst = ctx.enter_context(tc.tile_pool(name="cst", bufs=1))
        big = ctx.enter_context(tc.tile_pool(name="big", bufs=1))

        dis_sb = cst.tile([128, NCOLS], f32, tag="dis")
        nc.sync.dma_start(out=dis_sb[:], in_=disp[:, :])
        b2t = cst.tile([128, OUT], f32, tag="b2t")
        nc.sync.dma_start(out=b2t[:], in_=b2b[:, :])

        _common_agg(nc, bass, tile, mybir, ctx, tc, calls, tabs, idx0g, idxh,
                    repb, idxsc,
                    F=OUT, stage_dt=f32, gather_elem=OUT, gather_step=64,
                    tab_col_of=lambda q: tabs[q][:, :OUT],
                    acc=acc, acc_step=64, NQ=NQ, split_scatter=True,
                    no_scatter=no_scatter, no_reduce=no_reduce,
                    astrip_bufs=4, stage_bufs=8)

        # ---- epilogue: out = (S2 + self)*dis + b2; read only the 2 used
        # f32 of each 64-wide acc row (8B strided elems ride the 7ns floor)
        if skip_epi:
            nc.compile()
            return nc
        sS = big.tile([128, NCOLS * OUT], f32, tag="sS")
        accb = acc[:, :]
        nc.sync.dma_start(
            out=sS[:].rearrange("p (m f) -> p m f", m=NCOLS),
            in_=bass.AP(accb.tensor, accb.offset,
                        [[NCOLS * 64, 128], [64, NCOLS], [1, OUT]]))
        selft = big.tile([128, NCOLS * OUT], f32, tag="selft")
        nc.sync.dma_start(out=selft[:], in_=selft2[:, :])
        nc.vector.tensor_tensor(
            out=sS[:].rearrange("p (m f) -> p m f", m=NCOLS),
            in0=sS[:].rearrange("p (m f) -> p m f", m=NCOLS),
            in1=selft[:].rearrange("p (m f) -> p m f", m=NCOLS),
            op=mybir.AluOpType.add)
        dview = bass.AP(dis_sb.tensor, dis_sb[:].offset,
                        [dis_sb[:].ap[0], [1, NCOLS], [0, OUT]])
        sv = bass.AP(sS.tensor, sS[:].offset,
                     [sS[:].ap[0], [OUT, NCOLS], [1, OUT]])
        b2view = bass.AP(b2t.tensor, b2t[:].offset,
                         [b2t[:].ap[0], [0, NCOLS], [1, OUT]])
        nc.vector.tensor_tensor(out=sv, in0=sv, in1=dview,
                                op=mybir.AluOpType.mult)
        nc.vector.tensor_tensor(out=sv, in0=sv, in1=b2view,
                                op=mybir.AluOpType.add)
        nc.sync.dma_start(out=out[:, :], in_=sS[:])
    nc.compile()
    return nc


def _pmajor(arr_pad):
    """[PC_PAD, F] node order -> [128, NCOLS*F] p-major."""
    F = arr_pad.shape[1]
    return np.ascontiguousarray(
        arr_pad.reshape(NCOLS, 128, F).transpose(1, 0, 2).reshape(128, NCOLS * F))


def _unpmajor(arr_pm, F):
    """[128, NCOLS*F] p-major -> [PC_PAD, F] node order."""
    return np.ascontiguousarray(
        arr_pm.reshape(128, NCOLS, F).transpose(1, 0, 2).reshape(PC_PAD, F))


def kernel(x, edge_index, W1, b1, W2, b2):
    import ml_dtypes
    bf16 = ml_dtypes.bfloat16
    x = np.asarray(x, dtype=np.float32)
    W1 = np.asarray(W1, dtype=np.float32)
    b1 = np.asarray(b1, dtype=np.float32)
    W2 = np.asarray(W2, dtype=np.float32)
    b2 = np.asarray(b2, dtype=np.float32)

    bass, bacc, tile, mybir, run_spmd = _bass_mods()

    dis, (c1, x1, s1, nd1), (c2, x2, s2, nd2) = _host_prep(edge_index)
    cores = list(range(NC))

    # idx broadcast operands: hi/lo bf16 rows of the 16-partition wrap, and
    # the stacked replication matrix (256*rep | rep)
    def _idx_ops(xg, calls):
        base = xg[:, :16, :].astype(np.int32)   # [NC, 16, cols]
        idxh = np.concatenate([base // 256, base % 256], axis=1).astype(bf16)
        c0 = sum(calls[0][1]) * 8
        idx0g = np.ascontiguousarray(xg[:, :, :c0])
        return idxh, idx0g

    repb = np.zeros((32, 128), bf16)
    for p in range(128):
        repb[p % 16, p] = 256.0
        repb[16 + p % 16, p] = 1.0

    def _dpad(i):
        dp = np.concatenate([dis[i * PERCORE:(i + 1) * PERCORE],
                             np.ones(PC_PAD - PERCORE, np.float32)])
        return dp

    disps = [np.ascontiguousarray(_dpad(i).reshape(NCOLS, 128).T)
             for i in cores]

    # ---- launch 1: mm ----
    nc1 = _build_mm()
    in1 = []
    for i in cores:
        xT = np.zeros((E_CH, PC_PAD), bf16)
        xT[:, :PERCORE] = x[i * PERCORE:(i + 1) * PERCORE].T.astype(bf16)
        in1.append({"xT": xT, "W1": W1.astype(bf16), "disp": disps[i]})
    r1 = run_spmd(nc1, in1, core_ids=cores)
    h1p = np.concatenate([
        _unpmajor(np.asarray(r1.results[i]["out"]), HID)[:PERCORE]
        for i in cores])  # [N, HID] bf16

    # ---- host: pack layer-1 pair tables ----
    tabs1 = []
    for c in range(2):
        t = np.zeros((L1_ROWS, 128), bf16)
        t[:25000] = h1p[c * 50000:(c + 1) * 50000].reshape(25000, 128)
        tabs1.append(t)

    # ---- launch 2 ----
    nc2 = _build_agg1(c1, x1.shape[2])
    idxh1, idx0g1 = _idx_ops(x1, c1)
    in2 = []
    for i in cores:
        dp = np.concatenate([dis[i * PERCORE:(i + 1) * PERCORE],
                             np.ones(PC_PAD - PERCORE, np.float32)])
        # fold the bias in: a1 = relu(dis^2*(S + selfh + b1/dis)) on device
        h1pad = np.zeros((PC_PAD, HID), np.float32)
        h1pad[:PERCORE] = h1p[i * PERCORE:(i + 1) * PERCORE].astype(np.float32)
        h1pad += b1[None, :] / dp[:, None]
        m = {f"tab{c}": tabs1[c] for c in range(2)}
        m.update({
            "idxh": idxh1[i],
            "idx0g": idx0g1[i],
            "repb": repb,
            "idxsc": np.concatenate([s1[i, q] for q in range(NPASS)], axis=1),
            # epilogue constant: dis^2 per destination
            "disp": np.ascontiguousarray((dp * dp).reshape(NCOLS, 128).T),
            "selfh": _pmajor(h1pad.astype(bf16)),
            "W2": W2.astype(bf16),
        })
        in2.append(m)
    r2 = run_spmd(nc2, in2, core_ids=cores)
    t2 = np.concatenate([
        _unpmajor(np.asarray(r2.results[i]["out"]), OUT)[:PERCORE]
        for i in cores])  # [N, 2] f32

    # ---- host: pack layer-2 tables ----
    tabs2 = []
    for c in range(NPASS):
        t = np.zeros((L2_ROWS, 64), np.float32)
        t[:25000, :OUT] = t2[c * 25000:(c + 1) * 25000]
        tabs2.append(t)

    # ---- launch 3 ----
    nc3 = _build_agg2(c2, x2.shape[2])
    idxh2, idx0g2 = _idx_ops(x2, c2)
    b2bc = np.broadcast_to(b2, (128, OUT)).astype(np.float32).copy()
    in3 = []
    for i in cores:
        t2pad = np.zeros((PC_PAD, OUT), np.float32)
        t2pad[:PERCORE] = t2[i * PERCORE:(i + 1) * PERCORE]
        m = {f"tab{c}": tabs2[c] for c in range(NPASS)}
        m.update({
            "idxh": idxh2[i],
            "idx0g": idx0g2[i],
            "repb": repb,
            "idxsc": np.concatenate([s2[i, q] for q in range(NPASS)], axis=1),
            "disp": disps[i],
            "b2b": b2bc,
            "selft2": _pmajor(t2pad),
        })
        in3.append(m)
    r3 = run_spmd(nc3, in3, core_ids=cores)
    outv = np.concatenate([
        _unpmajor(np.asarray(r3.results[i]["out"]), OUT)[:PERCORE]
        for i in cores])
    return outv.astype(np.float32)



# revision 79
# speedup vs baseline: 1.0171x; 1.0017x over previous
"""Two-layer GCN on 8 Trainium2 NeuronCores — v2.1 (descriptor-lean).

HW is SWDGE-descriptor-rate bound (~4-5ns/desc at 4 queues, size-independent),
so v2.1 minimizes descriptor COUNT beyond the one-desc-per-edge gather:
- Self-loops never gathered: own-shard h1p/t2 terms are bulk-loaded p-major
  (~128 descs) and added on DVE.
- Per-pass partials scatter-add (f32, dense 256B rows) into a p-major
  accumulator keyed by true destination, so the epilogue re-reads the whole
  accumulator with 128 descriptors (one contiguous run per partition).
- All launch inputs/outputs that are per-destination use the plain p-major
  [128, 98*F] layout (dst = n*128 + p at [p, n*F:(n+1)*F]) — bulk DMA.
Layer tables as in v2: layer-1 bf16 pair-packed rows (128B descs via the
256B-stride sub-row gather), layer-2 f32 2-wide (8B descs). Gather perms are
per-pass global degree sorts (tight ~5% slot padding).
"""
import numpy as np

N = 100000
E_CH = 128
HID = 64
OUT = 2
NC = 8
PERCORE = 12500
PC_PAD = 12544
NCOLS = 98               # p-major columns (dst = n*128+p, n in [0,98))
NPASS = 4
L1_ROWS = 25001
L2_ROWS = 25001
ZROW = 25000
MAXPOS = 16384           # gather positions per call (slot-major k-blocks);
                         # stage tile is [128, (MAXPOS//128)*F]
STAGE_COLS = MAXPOS // 128


def _wrap_idx(vals):
    ni = len(vals)
    assert ni % 16 == 0
    return np.tile(vals.reshape(ni // 16, 16).T, (8, 1))


def _prep_layer(src, dst, pass_of, row_of):
    """Per-layer prep: per-pass degree-sorted perms, per-tile shared-K slot
    matrices, slot-major (k-block) gather calls, gather idx, and p-major
    scatter idx. No self-loops here.

    Slot-major: positions are ordered (k, tile, p) with c_k = #tiles whose
    K exceeds k. Each call covers consecutive k's with sum(c_k)*128 <=
    MAXPOS, so slot padding is per-tile-max only (~3%) and the reduce is a
    prefix-add tree on DVE (packed bf16/f32, 2x mode) instead of a strided
    tensor_reduce."""
    p = pass_of(src)
    r = row_of(src)
    core = dst // PERCORE
    dloc = dst % PERCORE

    cnt = np.bincount(dst * NPASS + p, minlength=N * NPASS).reshape(N, NPASS)

    perms = np.zeros((NC, NPASS, PC_PAD), np.int64)
    K = np.zeros((NPASS, PC_PAD // 128), np.int64)
    for i in range(NC):
        c0 = cnt[i * PERCORE:(i + 1) * PERCORE]
        for q in range(NPASS):
            pp = np.argsort(-c0[:, q], kind="stable")
            pp = np.concatenate([pp, np.arange(PERCORE, PC_PAD)])
            perms[i, q] = pp
            cc = np.concatenate([c0[:, q], np.zeros(PC_PAD - PERCORE, np.int64)])
            kt = cc[pp].reshape(-1, 128).max(axis=1)
            K[q] = np.maximum(K[q], kt)
    K = np.maximum(K, 1)

    invperms = np.zeros((NC, NPASS, PC_PAD), np.int64)
    for i in range(NC):
        for q in range(NPASS):
            invperms[i, q][perms[i, q]] = np.arange(PC_PAD)
    Ms = [[None] * NPASS for _ in range(NC)]
    for i in range(NC):
        ec = core == i
        for q in range(NPASS):
            sel = ec & (p == q)
            ed = dloc[sel]
            es = r[sel]
            order = np.argsort(ed, kind="stable")
            ed = ed[order]
            es = es[order]
            starts = np.searchsorted(ed, np.arange(PERCORE))
            rank = np.arange(len(ed)) - starts[ed]
            kmax = int(K[q].max())
            M = np.full((PC_PAD, kmax), ZROW, np.int16)
            M[invperms[i, q][ed], rank] = es.astype(np.int16)
            Ms[i][q] = M

    # slot-major call packing: per pass, greedy k-blocks under MAXPOS
    calls = []  # (pass, [c_k, ...], k0, tile0)
    for q in range(NPASS):
        kmax = int(K[q].max())
        cs_all = [int((K[q] > k).sum()) for k in range(kmax)]
        cur = []
        k0 = 0
        for ck in cs_all:
            if cur and (sum(cur) + ck) * 128 > MAXPOS:
                calls.append((q, cur, k0, 0))
                k0 += len(cur)
                cur = []
            cur.append(ck)
        if cur:
            calls.append((q, cur, k0, 0))
    # halve the very first call: its descgen (~5.3us) gates the launch ramp
    q0, cs0, _, _ = calls[0]
    if len(cs0) == 1 and cs0[0] > 2:
        a = cs0[0] // 3
        b = 2 * (cs0[0] // 3)
        calls[0:1] = [(q0, [a], 0, 0), (q0, [b - a], 0, a),
                      (q0, [cs0[0] - b], 0, b)]

    idxg = []
    for i in range(NC):
        parts = []
        for (q, cs, k0, t0) in calls:
            vals = np.concatenate(
                [Ms[i][q][t0 * 128:(t0 + ck) * 128, k0 + j].reshape(ck, 128)
                 for j, ck in enumerate(cs)], axis=0)
            parts.append(_wrap_idx(vals.ravel()))
        idxg.append(np.concatenate(parts, axis=1))
    idxg = np.stack(idxg)  # [NC, 128, COLS_G]

    # scatter idx per (core, pass): perm position i -> p-major acc row of the
    # true dst; pad positions (perm rank >= PERCORE, trailing) -> -1
    idxsc = np.zeros((NC, NPASS, 128, PC_PAD // 16), np.int16)
    for i in range(NC):
        for q in range(NPASS):
            d = perms[i, q]
            v = ((d % 128) * NCOLS + d // 128).astype(np.int16)
            v[PERCORE:] = -1
            idxsc[i, q] = _wrap_idx(v)

    ndesc = int(K.sum()) * 128
    return calls, idxg, idxsc, ndesc


def _host_prep(edge_index):
    src = np.asarray(edge_index[0], dtype=np.int64)
    dst = np.asarray(edge_index[1], dtype=np.int64)
    deg = np.bincount(dst, minlength=N).astype(np.float64) + 1.0
    dis = (1.0 / np.sqrt(deg)).astype(np.float32)

    # no appended self-loops; added directly in the epilogues
    l1 = _prep_layer(src, dst,
                     lambda s: (s // 50000) * 2 + (s % 2),
                     lambda s: (s % 50000) // 2)
    l2 = _prep_layer(src, dst,
                     lambda s: s // 25000,
                     lambda s: s % 25000)
    return dis, l1, l2


def _bass_mods():
    import sys
    if "/opt/trn_rl_repo" not in sys.path:
        sys.path.insert(0, "/opt/trn_rl_repo")
    import concourse.bass as bass
    import concourse.bacc as bacc
    import concourse.tile as tile
    from concourse import mybir
    from concourse.bass_utils import run_bass_kernel_spmd
    return bass, bacc, tile, mybir, run_bass_kernel_spmd


def _dma_gather_thin(gp, out_ap, in_ap, idxs_ap, num_idxs, elem_size,
                     elem_step, queue_num):
    from concourse import mybir
    gp._assert_queue_num(queue_num)
    assert idxs_ap.dtype == mybir.dt.int16
    stride_bytes = elem_step * mybir.dt.size(in_ap.dtype)
    assert stride_bytes % 256 == 0 and stride_bytes // 256 < 256
    assert in_ap.ap[-1][1] == elem_size
    assert in_ap.ap[0][0] == elem_step
    _in_ap = gp.lower_ap_dma(in_ap, for_custom_bir_dma=True)
    _idxs_ap = gp.lower_ap(idxs_ap)
    _out_ap = gp.lower_ap(out_ap)
    return gp.add_instruction(
        mybir.InstDMAGatherAnt(
            name=gp.bass.get_next_instruction_name(),
            ins=[*_in_ap, _idxs_ap, gp.lower_val_access(gp.to_reg(num_idxs))],
            outs=[_out_ap],
            transpose=False,
            num_idxs=num_idxs,
            elem_size=elem_size,
            stride_bytes_256=stride_bytes // 256,
            gen_mode=0,
            single_packet=False,
            queue_num=queue_num,
            sbuf_tokens_per_rank=0,
            sbuf_free_dim_per_rank=0,
            sbuf_free_dim_pad_per_rank=0,
            sbuf_byte_offset=0,
        )
    )


def _build_mm():
    """h1p = (x @ W1) * dis for own shard, bf16, p-major output."""
    bass, bacc, tile, mybir, _ = _bass_mods()
    from contextlib import ExitStack
    nc = bacc.Bacc()
    bf = mybir.dt.bfloat16
    xT = nc.declare_dram_parameter("xT", [E_CH, PC_PAD], bf, isOutput=False)
    W1 = nc.declare_dram_parameter("W1", [E_CH, HID], bf, isOutput=False)
    disp = nc.declare_dram_parameter("disp", [128, NCOLS], mybir.dt.float32,
                                     isOutput=False)
    out = nc.declare_dram_parameter("out", [128, NCOLS * HID], bf,
                                    isOutput=True)
    G = 14
    with tile.TileContext(nc) as tc, ExitStack() as ctx:
        wp = ctx.enter_context(tc.tile_pool(name="wp", bufs=1))
        sb = ctx.enter_context(tc.tile_pool(name="sb", bufs=3))
        ps = ctx.enter_context(tc.tile_pool(name="ps", bufs=4, space="PSUM"))
        w1 = wp.tile([E_CH, HID], bf, tag="w1")
        nc.sync.dma_start(out=w1[:], in_=W1[:, :])
        dis_sb = wp.tile([128, NCOLS], mybir.dt.float32, tag="dis")
        nc.sync.dma_start(out=dis_sb[:], in_=disp[:, :])
        PB = 7
        for g in range(0, NCOLS, G):
            ng = min(G, NCOLS - g)
            xt = sb.tile([E_CH, G * 128], bf, tag="xt")
            nc.sync.dma_start(out=xt[:, :ng * 128],
                              in_=xT[:, g * 128:(g + ng) * 128])
            ot = sb.tile([128, G * HID], bf, tag="ot")
            for h0 in range(0, ng, PB):
                nh = min(PB, ng - h0)
                pt = ps.tile([128, PB * HID], mybir.dt.float32, space="PSUM",
                             tag="pt")
                for j in range(nh):
                    nc.tensor.matmul(pt[:, j * HID:(j + 1) * HID],
                                     lhsT=xt[:, (h0 + j) * 128:
                                             (h0 + j + 1) * 128],
                                     rhs=w1[:], start=True, stop=True)
                # scale each column's HID block by its dis in one strided op
                dview = bass.AP(dis_sb.tensor, dis_sb[:].offset + g + h0,
                                [dis_sb[:].ap[0], [1, nh], [0, HID]])
                pv = bass.AP(pt.tensor, pt[:].offset,
                             [pt[:].ap[0], [HID, nh], [1, HID]])
                ov = bass.AP(ot.tensor, ot[:].offset + h0 * HID,
                             [ot[:].ap[0], [HID, nh], [1, HID]])
                with nc.allow_low_precision(reason="bf16 h1 staging"):
                    nc.vector.tensor_tensor(out=ov, in0=pv, in1=dview,
                                            op=mybir.AluOpType.mult)
            nc.sync.dma_start(out=out[:, g * HID:(g + ng) * HID],
                              in_=ot[:, :ng * HID])
    nc.compile()
    return nc


def _common_agg(nc, bass, tile, mybir, ctx, tc, calls, tabs, idx0g, idxh,
                repb, idxsc, F, stage_dt, gather_elem, gather_step,
                tab_col_of, acc, acc_step, NQ, split_scatter=True,
                no_scatter=False, no_reduce=False, astrip_bufs=2,
                stage_bufs=4):
    """Shared gather/reduce/scatter pipeline. astrip is compact
    [128, NCOLS*F] in stage_dt; scatter writes F elems per destination into
    `acc` (row stride acc_step elems = 256B; untouched columns stay zero via
    output zero-donation).

    Gather indices arrive as hi/lo bf16 [32, cols] (idxh) and are broadcast
    to the q7-required 8x-replicated int16 [128, cols] layout on-chip:
    PE matmul against repb (256*rep | rep) then an exact f32->int16 convert
    on DVE. This cuts idx HBM traffic 4x. Call 0 uses a small direct int16
    load (idx0g) so the first gather isn't gated on the broadcast pipeline."""
    ib = ctx.enter_context(tc.tile_pool(name="ib", bufs=2))
    stp = ctx.enter_context(tc.tile_pool(name="stp", bufs=stage_bufs))
    ap_ = ctx.enter_context(tc.tile_pool(name="ap", bufs=astrip_bufs))
    psp = ctx.enter_context(tc.tile_pool(name="psp", bufs=2, space="PSUM"))

    cst_local = ctx.enter_context(tc.tile_pool(name="cstl", bufs=1))
    SCC = PC_PAD // 16
    iscb = cst_local.tile([128, NPASS * SCC], mybir.dt.int16, tag="iscb")
    iscb_loaded = [False]
    repb_sb = cst_local.tile([32, 128], mybir.dt.bfloat16, tag="repb")
    nc.sync.dma_start(out=repb_sb[:], in_=repb[:, :])
    BCH = 512  # psum-chunk columns per broadcast matmul

    def ensure_iscb():
        # deferred so the launch ramp isn't spent on scatter indices
        if not iscb_loaded[0]:
            nc.sync.dma_start(out=iscb[:], in_=idxsc[:, :])
            iscb_loaded[0] = True

    qn = 0
    goff = 0
    cur_pass = -1
    idx_sb = None
    idx0_sb = None
    pass_goff = 0
    astrip = None
    pass_cols = {}
    pass_ncalls = {}
    for (q, cs, _k0, _t0) in calls:
        pass_cols[q] = pass_cols.get(q, 0) + sum(cs) * 8
        pass_ncalls[q] = pass_ncalls.get(q, 0) + 1

    HCOL = NCOLS // 2          # 49 astrip columns per scatter half
    HPOS = HCOL * 128          # 6272 positions per half

    # per pass: index (within the pass) of the last call touching any tile
    # >= HCOL; after it, astrip cols [HCOL, NCOLS) are final (c_k shrinks)
    last_big = {}
    seen = {}
    for (q, cs, _k0, t0) in calls:
        j = seen.get(q, 0)
        if t0 + cs[0] > HCOL:
            last_big[q] = j
        seen[q] = j + 1

    def flush_half(q, astrip_t, half):
        if no_scatter:
            return
        ensure_iscb()
        base = astrip_t[:]
        if not split_scatter and half == 1:
            nc.gpsimd.dma_scatter_add(
                out_ap=acc[:, :F],
                in_ap=astrip_t[:].rearrange("p (k f) -> p k f", k=NCOLS),
                idxs_ap=iscb[:, q * SCC:(q + 1) * SCC],
                num_idxs=PC_PAD, num_idxs_reg=PERCORE,
                elem_size=F, elem_step=acc_step,
                queue_num=q % NQ, single_packet=False)
            return
        nc.gpsimd.dma_scatter_add(
            out_ap=acc[:, :F],
            in_ap=bass.AP(astrip_t.tensor, base.offset + half * HCOL * F,
                          [base.ap[0], [F, HCOL], [1, F]]),
            idxs_ap=iscb[:, q * SCC + half * (HPOS // 16):
                         q * SCC + (half + 1) * (HPOS // 16)],
            num_idxs=HPOS,
            num_idxs_reg=HPOS if half == 0 else PERCORE - HPOS,
            elem_size=F, elem_step=acc_step,
            queue_num=q % NQ, single_packet=False)

    # call-0 fast path: direct int16 load so gather 0 isn't gated on the
    # broadcast pipeline (its ~9us transfer then covers the convert latency)
    c0 = sum(calls[0][1]) * 8
    idx0_sb = cst_local.tile([128, c0], mybir.dt.int16, tag="idx0")
    nc.sync.dma_start(out=idx0_sb[:], in_=idx0g[:, :c0])

    # broadcast pipelines are emitted lookahead-1: pass q+1's converts land
    # on DVE between pass q's early reduce ops, so they neither stall the
    # next pass's gathers nor push the whole reduce/scatter chain late
    pass_off = [0] * NPASS
    go = 0
    for q in range(NPASS):
        pass_off[q] = go
        go += pass_cols[q]
    idx_tiles = [None] * NPASS

    def emit_idx_pipeline(q):
        ccols = pass_cols[q]
        idxh_sb = ib.tile([32, ccols], mybir.dt.bfloat16, tag="idxh")
        nc.sync.dma_start(out=idxh_sb[:],
                          in_=idxh[:, pass_off[q]:pass_off[q] + ccols])
        idx_sb = cst_local.tile([128, ccols], mybir.dt.int16, tag=f"idx{q}")
        for o in range(0, ccols, BCH):
            w = min(BCH, ccols - o)
            pidx = psp.tile([128, BCH], mybir.dt.float32, space="PSUM",
                            tag="pidx")
            nc.tensor.matmul(pidx[:, :w], lhsT=repb_sb[:],
                             rhs=idxh_sb[:, o:o + w],
                             start=True, stop=True)
            nc.vector.tensor_scalar_add(idx_sb[:, o:o + w],
                                        pidx[:, :w], 0.0)
        idx_tiles[q] = idx_sb

    emit_idx_pipeline(0)

    call_in_pass = 0
    for (q, cs, _k0, t0) in calls:
        if q != cur_pass:
            if astrip is not None:
                # high-degree half (cols [0, HCOL)) finalizes at pass end
                flush_half(cur_pass, astrip,
                           0 if split_scatter else 1)
            cur_pass = q
            pass_goff = goff
            call_in_pass = 0
            if idx_tiles[q] is None:
                emit_idx_pipeline(q)
            idx_sb = idx_tiles[q]
            astrip = ap_.tile([128, NCOLS * F], stage_dt, tag="astrip")
        tot = sum(cs)
        ni = tot * 128
        stage = stp.tile([128, STAGE_COLS * F], stage_dt, tag="stage")
        lo = goff - pass_goff
        if q == 0 and lo == 0:
            idx_view = idx0_sb[:, :tot * 8]
        else:
            idx_view = idx_sb[:, lo:lo + tot * 8]
        _dma_gather_thin(
            nc.gpsimd,
            out_ap=bass.AP(stage.tensor, stage[:].offset,
                           [stage[:].ap[0], [F, tot], [1, F]]),
            in_ap=tab_col_of(q),
            idxs_ap=idx_view,
            num_idxs=ni, elem_size=gather_elem, elem_step=gather_step,
            queue_num=qn)
        qn = (qn + 1) % NQ

        # prefix-add tree over the call's k-blocks (c nonincreasing), then
        # one add (or init copy) into astrip[0 : c_first*F)
        if no_reduce:
            goff += tot * 8
            call_in_pass += 1
            continue
        sap0 = stage[:].ap[0]
        soff = stage[:].offset
        blocks = []
        o = 0
        for ck in cs:
            blocks.append((o, ck))
            o += ck
        with nc.allow_low_precision(reason="short partial sums, tree depth"):
            while len(blocks) > 1:
                nxt = []
                for a in range(0, len(blocks) - 1, 2):
                    (o0, c0b), (o1, c1b) = blocks[a], blocks[a + 1]
                    v0 = bass.AP(stage.tensor, soff + o0 * F,
                                 [sap0, [F, c1b], [1, F]])
                    v1 = bass.AP(stage.tensor, soff + o1 * F,
                                 [sap0, [F, c1b], [1, F]])
                    nc.vector.tensor_tensor(out=v0, in0=v0, in1=v1,
                                            op=mybir.AluOpType.add)
                    nxt.append((o0, c0b))
                if len(blocks) % 2:
                    nxt.append(blocks[-1])
                blocks = nxt
            (o0, cfin) = blocks[0]
            srcap = bass.AP(stage.tensor, soff + o0 * F,
                            [sap0, [F, cfin], [1, F]])
            dstap = bass.AP(astrip.tensor, astrip[:].offset + t0 * F,
                            [astrip[:].ap[0], [F, cfin], [1, F]])
            if call_in_pass == 0 or t0 > 0:
                # c_0 == NCOLS (K >= 1 everywhere): initializes all of astrip
                nc.vector.tensor_scalar_add(out=dstap, in0=srcap, scalar1=0.0)
            else:
                nc.vector.tensor_tensor(out=dstap, in0=dstap, in1=srcap,
                                        op=mybir.AluOpType.add)
        goff += tot * 8
        if split_scatter and call_in_pass == last_big[q]:
            # low-degree half's tiles are never touched by later (smaller-c)
            # calls of this pass
            flush_half(q, astrip, 1)
        if call_in_pass == 1 and q + 1 < NPASS and idx_tiles[q + 1] is None:
            emit_idx_pipeline(q + 1)
        call_in_pass += 1
    flush_half(cur_pass, astrip, 0 if split_scatter else 1)


def _build_agg1(calls, cols_g, skip_epi=False, no_scatter=False,
                no_reduce=False):
    """Layer-1 aggregation + self add + epilogue t2 = relu(...) @ W2."""
    bass, bacc, tile, mybir, _ = _bass_mods()
    from contextlib import ExitStack
    from concourse.masks import make_identity
    bf = mybir.dt.bfloat16
    f32 = mybir.dt.float32
    NQ = 4
    nc = bacc.Bacc(num_swdge_queues=NQ, dynamic_dma_scratch_size=8192 * NQ)
    tabs = [nc.declare_dram_parameter(f"tab{c}", [L1_ROWS, 128], bf,
                                      isOutput=False) for c in range(2)]
    c0 = sum(calls[0][1]) * 8
    idx0g = nc.declare_dram_parameter("idx0g", [128, c0], mybir.dt.int16,
                                      isOutput=False)
    idxh = nc.declare_dram_parameter("idxh", [32, cols_g], bf, isOutput=False)
    repb = nc.declare_dram_parameter("repb", [32, 128], bf, isOutput=False)
    idxsc = nc.declare_dram_parameter("idxsc", [128, NPASS * (PC_PAD // 16)],
                                      mybir.dt.int16, isOutput=False)
    disp = nc.declare_dram_parameter("disp", [128, NCOLS], f32, isOutput=False)
    selfh = nc.declare_dram_parameter("selfh", [128, NCOLS * HID], bf,
                                      isOutput=False)
    W2 = nc.declare_dram_parameter("W2", [HID, OUT], bf, isOutput=False)
    acc = nc.declare_dram_parameter("acc", [PC_PAD, 128], bf, isOutput=True)
    out = nc.declare_dram_parameter("out", [128, NCOLS * OUT], f32,
                                    isOutput=True)

    with tile.TileContext(nc) as tc, ExitStack() as ctx:
        cst = ctx.enter_context(tc.tile_pool(name="cst", bufs=1))
        ep = ctx.enter_context(tc.tile_pool(name="ep", bufs=3))
        ps = ctx.enter_context(tc.tile_pool(name="ps", bufs=4, space="PSUM"))

        dis_sb = cst.tile([128, NCOLS], f32, tag="dis")
        nc.sync.dma_start(out=dis_sb[:], in_=disp[:, :])
        w2t = cst.tile([HID, OUT], bf, tag="w2t")
        nc.sync.dma_start(out=w2t[:], in_=W2[:, :])
        ident = cst.tile([128, 128], bf, tag="ident")
        make_identity(nc, ident[:])

        _common_agg(nc, bass, tile, mybir, ctx, tc, calls, tabs, idx0g, idxh,
                    repb, idxsc,
                    F=HID, stage_dt=bf, gather_elem=HID, gather_step=128,
                    tab_col_of=lambda q: tabs[q // 2][:, (q % 2) * HID:
                                                      (q % 2 + 1) * HID],
                    acc=acc, acc_step=128, NQ=NQ,
                    no_scatter=no_scatter, no_reduce=no_reduce)

        # ---- epilogue (GE-chunked reads of the p-major bf16 accumulator;
        # acc rows are 128-wide with cols HID..128 zero from donation).
        # selfh comes in with b1/dis pre-folded on host, so
        # a1 = relu(dis^2 * (S + selfh)); PSUM work is batched PB columns per
        # ACT copy to amortize the ~370ns scalar-engine access latency. ----
        GE = 10
        PB = 5
        for g0 in ([] if skip_epi else range(0, NCOLS, GE)):
            ng = min(GE, NCOLS - g0)
            sS = ep.tile([128, GE * 128], bf, tag="sS")
            accb = acc[:, :]
            nc.sync.dma_start(
                out=sS[:, :ng * 128].rearrange("p (m f) -> p m f", m=ng),
                in_=bass.AP(accb.tensor, accb.offset + g0 * 128,
                            [[NCOLS * 128, 128], [128, ng], [1, 128]]))
            selft = ep.tile([128, GE * HID], bf, tag="selft")
            nc.sync.dma_start(out=selft[:, :ng * HID],
                              in_=selfh[:, g0 * HID:(g0 + ng) * HID])
            svs = bass.AP(sS.tensor, sS[:].offset,
                          [sS[:].ap[0], [128, ng], [1, HID]])
            selfv = bass.AP(selft.tensor, selft[:].offset,
                            [selft[:].ap[0], [HID, ng], [1, HID]])
            with nc.allow_low_precision(reason="bf16 self add"):
                nc.vector.tensor_tensor(out=svs, in0=svs, in1=selfv,
                                        op=mybir.AluOpType.add)
            a1 = ep.tile([128, GE * HID], bf, tag="a1")
            dview = bass.AP(dis_sb.tensor, dis_sb[:].offset + g0,
                            [dis_sb[:].ap[0], [1, ng], [0, HID]])
            sv = bass.AP(sS.tensor, sS[:].offset,
                         [sS[:].ap[0], [128, ng], [1, HID]])
            av = bass.AP(a1.tensor, a1[:].offset,
                         [a1[:].ap[0], [HID, ng], [1, HID]])
            with nc.allow_low_precision(reason="bf16 epilogue"):
                nc.vector.tensor_tensor(out=av, in0=sv, in1=dview,
                                        op=mybir.AluOpType.mult)
                nc.vector.tensor_scalar_max(a1[:, :ng * HID],
                                            a1[:, :ng * HID], 0.0)
            ostrip = ep.tile([128, GE * OUT], f32, tag="ostrip")
            for h0 in range(0, ng, PB):
                nh = min(PB, ng - h0)
                putb = ps.tile([HID, PB * 128], bf, space="PSUM", tag="putb")
                for j in range(nh):
                    nc.tensor.transpose(
                        out=putb[:, j * 128:(j + 1) * 128],
                        in_=a1[:, (h0 + j) * HID:(h0 + j + 1) * HID],
                        identity=ident[:])
                utb = ep.tile([HID, PB * 128], bf, tag="utb")
                nc.scalar.activation(out=utb[:, :nh * 128],
                                     in_=putb[:, :nh * 128],
                                     func=mybir.ActivationFunctionType.Copy)
                pob = ps.tile([128, PB * OUT], f32, space="PSUM", tag="pob")
                for j in range(nh):
                    nc.tensor.matmul(pob[:, j * OUT:(j + 1) * OUT],
                                     lhsT=utb[:, j * 128:(j + 1) * 128],
                                     rhs=w2t[:], start=True, stop=True)
                nc.scalar.activation(out=ostrip[:, h0 * OUT:(h0 + nh) * OUT],
                                     in_=pob[:, :nh * OUT],
                                     func=mybir.ActivationFunctionType.Copy)
            nc.sync.dma_start(out=out[:, g0 * OUT:(g0 + ng) * OUT],
                              in_=ostrip[:, :ng * OUT])
    nc.compile()
    return nc


def _build_agg2(calls, cols_g, skip_epi=False, no_scatter=False,
                no_reduce=False):
    """Layer-2 aggregation of 2-wide f32 + self add + S2*dis + b2."""
    bass, bacc, tile, mybir, _ = _bass_mods()
    from contextlib import ExitStack
    f32 = mybir.dt.float32
    NQ = 4
    nc = bacc.Bacc(num_swdge_queues=NQ, dynamic_dma_scratch_size=8192 * NQ)
    tabs = [nc.declare_dram_parameter(f"tab{c}", [L2_ROWS, 64], f32,
                                      isOutput=False) for c in range(NPASS)]
    bf = mybir.dt.bfloat16
    c0 = sum(calls[0][1]) * 8
    idx0g = nc.declare_dram_parameter("idx0g", [128, c0], mybir.dt.int16,
                                      isOutput=False)
    idxh = nc.declare_dram_parameter("idxh", [32, cols_g], bf, isOutput=False)
    repb = nc.declare_dram_parameter("repb", [32, 128], bf, isOutput=False)
    idxsc = nc.declare_dram_parameter("idxsc", [128, NPASS * (PC_PAD // 16)],
                                      mybir.dt.int16, isOutput=False)
    disp = nc.declare_dram_parameter("disp", [128, NCOLS], f32, isOutput=False)
    b2b = nc.declare_dram_parameter("b2b", [128, OUT], f32, isOutput=False)
    selft2 = nc.declare_dram_parameter("selft2", [128, NCOLS * OUT], f32,
                                       isOutput=False)
    acc = nc.declare_dram_parameter("acc", [PC_PAD, 64], f32, isOutput=True)
    out = nc.declare_dram_parameter("out", [128, NCOLS * OUT], f32,
                                    isOutput=True)

    with tile.TileContext(nc) as tc, ExitStack() as ctx:
        cst = ctx.enter_context(tc.tile_pool(name="cst", bufs=1))
        big = ctx.enter_context(tc.tile_pool(name="big", bufs=1))

        dis_sb = cst.tile([128, NCOLS], f32, tag="dis")
        nc.sync.dma_start(out=dis_sb[:], in_=disp[:, :])
        b2t = cst.tile([128, OUT], f32, tag="b2t")
        nc.sync.dma_start(out=b2t[:], in_=b2b[:, :])

        _common_agg(nc, bass, tile, mybir, ctx, tc, calls, tabs, idx0g, idxh,
                    repb, idxsc,
                    F=OUT, stage_dt=f32, gather_elem=OUT, gather_step=64,
                    tab_col_of=lambda q: tabs[q][:, :OUT],
                    acc=acc, acc_step=64, NQ=NQ, split_scatter=True,
                    no_scatter=no_scatter, no_reduce=no_reduce,
                    astrip_bufs=4, stage_bufs=8)

        # ---- epilogue: out = (S2 + self)*dis + b2; read only the 2 used
        # f32 of each 64-wide acc row (8B strided elems ride the 7ns floor)
        if skip_epi:
            nc.compile()
            return nc
        sS = big.tile([128, NCOLS * OUT], f32, tag="sS")
        accb = acc[:, :]
        nc.sync.dma_start(
            out=sS[:].rearrange("p (m f) -> p m f", m=NCOLS),
            in_=bass.AP(accb.tensor, accb.offset,
                        [[NCOLS * 64, 128], [64, NCOLS], [1, OUT]]))
        selft = big.tile([128, NCOLS * OUT], f32, tag="selft")
        nc.sync.dma_start(out=selft[:], in_=selft2[:, :])
        nc.vector.tensor_tensor(
            out=sS[:].rearrange("p (m f) -> p m f", m=NCOLS),
            in0=sS[:].rearrange("p (m f) -> p m f", m=NCOLS),
            in1=selft[:].rearrange("p (m f) -> p m f", m=NCOLS),
            op=mybir.AluOpType.add)
        dview = bass.AP(dis_sb.tensor, dis_sb[:].offset,
                        [dis_sb[:].ap[0], [1, NCOLS], [0, OUT]])
        sv = bass.AP(sS.tensor, sS[:].offset,
                     [sS[:].ap[0], [OUT, NCOLS], [1, OUT]])
        b2view = bass.AP(b2t.tensor, b2t[:].offset,
                         [b2t[:].ap[0], [0, NCOLS], [1, OUT]])
        nc.vector.tensor_tensor(out=sv, in0=sv, in1=dview,
                                op=mybir.AluOpType.mult)
        nc.vector.tensor_tensor(out=sv, in0=sv, in1=b2view,
                                op=mybir.AluOpType.add)
        nc.sync.dma_start(out=out[:, :], in_=sS[:])
    nc.compile()
    return nc


def _pmajor(arr_pad):
    """[PC_PAD, F] node order -> [128, NCOLS*F] p-major."""
    F = arr_pad.shape[1]
    return np.ascontiguousarray(
        arr_pad.reshape(NCOLS, 128, F).transpose(1, 0, 2).reshape(128, NCOLS * F))


def _unpmajor(arr_pm, F):
    """[128, NCOLS*F] p-major -> [PC_PAD, F] node order."""
    return np.ascontiguousarray(
        arr_pm.reshape(128, NCOLS, F).transpose(1, 0, 2).reshape(PC_PAD, F))


def kernel(x, edge_index, W1, b1, W2, b2):
    import ml_dtypes
    bf16 = ml_dtypes.bfloat16
    x = np.asarray(x, dtype=np.float32)
    W1 = np.asarray(W1, dtype=np.float32)
    b1 = np.asarray(b1, dtype=np.float32)
    W2 = np.asarray(W2, dtype=np.float32)
    b2 = np.asarray(b2, dtype=np.float32)

    bass, bacc, tile, mybir, run_spmd = _bass_mods()

    dis, (c1, x1, s1, nd1), (c2, x2, s2, nd2) = _host_prep(edge_index)
    cores = list(range(NC))

    # idx broadcast operands: hi/lo bf16 rows of the 16-partition wrap, and
    # the stacked replication matrix (256*rep | rep)
    def _idx_ops(xg, calls):
        base = xg[:, :16, :].astype(np.int32)   # [NC, 16, cols]
        idxh = np.concatenate([base // 256, base % 256], axis=1).astype(bf16)
        c0 = sum(calls[0][1]) * 8
        idx0g = np.ascontiguousarray(xg[:, :, :c0])
        return idxh, idx0g

    repb = np.zeros((32, 128), bf16)
    for p in range(128):
        repb[p % 16, p] = 256.0
        repb[16 + p % 16, p] = 1.0

    def _dpad(i):
        dp = np.concatenate([dis[i * PERCORE:(i + 1) * PERCORE],
                             np.ones(PC_PAD - PERCORE, np.float32)])
        return dp

    disps = [np.ascontiguousarray(_dpad(i).reshape(NCOLS, 128).T)
             for i in cores]

    # ---- launch 1: mm ----
    nc1 = _build_mm()
    in1 = []
    for i in cores:
        xT = np.zeros((E_CH, PC_PAD), bf16)
        xT[:, :PERCORE] = x[i * PERCORE:(i + 1) * PERCORE].T.astype(bf16)
        in1.append({"xT": xT, "W1": W1.astype(bf16), "disp": disps[i]})
    r1 = run_spmd(nc1, in1, core_ids=cores)
    h1p = np.concatenate([
        _unpmajor(np.asarray(r1.results[i]["out"]), HID)[:PERCORE]
        for i in cores])  # [N, HID] bf16

    # ---- host: pack layer-1 pair tables ----
    tabs1 = []
    for c in range(2):
        t = np.zeros((L1_ROWS, 128), bf16)
        t[:25000] = h1p[c * 50000:(c + 1) * 50000].reshape(25000, 128)
        tabs1.append(t)

    # ---- launch 2 ----
    nc2 = _build_agg1(c1, x1.shape[2])
    idxh1, idx0g1 = _idx_ops(x1, c1)
    in2 = []
    for i in cores:
        dp = np.concatenate([dis[i * PERCORE:(i + 1) * PERCORE],
                             np.ones(PC_PAD - PERCORE, np.float32)])
        # fold the bias in: a1 = relu(dis^2*(S + selfh + b1/dis)) on device
        h1pad = np.zeros((PC_PAD, HID), np.float32)
        h1pad[:PERCORE] = h1p[i * PERCORE:(i + 1) * PERCORE].astype(np.float32)
        h1pad += b1[None, :] / dp[:, None]
        m = {f"tab{c}": tabs1[c] for c in range(2)}
        m.update({
            "idxh": idxh1[i],
            "idx0g": idx0g1[i],
            "repb": repb,
            "idxsc": np.concatenate([s1[i, q] for q in range(NPASS)], axis=1),
            # epilogue constant: dis^2 per destination
            "disp": np.ascontiguousarray((dp * dp).reshape(NCOLS, 128).T),
            "selfh": _pmajor(h1pad.astype(bf16)),
            "W2": W2.astype(bf16),
        })
        in2.append(m)
    r2 = run_spmd(nc2, in2, core_ids=cores)
    t2 = np.concatenate([
        _unpmajor(np.asarray(r2.results[i]["out"]), OUT)[:PERCORE]
        for i in cores])  # [N, 2] f32

    # ---- host: pack layer-2 tables ----
    tabs2 = []
    for c in range(NPASS):
        t = np.zeros((L2_ROWS, 64), np.float32)
        t[:25000, :OUT] = t2[c * 25000:(c + 1) * 25000]
        tabs2.append(t)

    # ---- launch 3 ----
    nc3 = _build_agg2(c2, x2.shape[2])
    idxh2, idx0g2 = _idx_ops(x2, c2)
    b2bc = np.broadcast_to(b2, (128, OUT)).astype(np.float32).copy()
    in3 = []
    for i in cores:
        t2pad = np.zeros((PC_PAD, OUT), np.float32)
        t2pad[:PERCORE] = t2[i * PERCORE:(i + 1) * PERCORE]
        m = {f"tab{c}": tabs2[c] for c in range(NPASS)}
        m.update({
            "idxh": idxh2[i],
            "idx0g": idx0g2[i],
            "repb": repb,
            "idxsc": np.concatenate([s2[i, q] for q in range(NPASS)], axis=1),
            "disp": disps[i],
            "b2b": b2bc,
            "selft2": _pmajor(t2pad),
        })
        in3.append(m)
    r3 = run_spmd(nc3, in3, core_ids=cores)
    outv = np.concatenate([
        _unpmajor(np.asarray(r3.results[i]["out"]), OUT)[:PERCORE]
        for i in cores])
    return outv.astype(np.float32)



# revision 82
# speedup vs baseline: 1.0184x; 1.0012x over previous
"""Two-layer GCN on 8 Trainium2 NeuronCores — v2.1 (descriptor-lean).

HW is SWDGE-descriptor-rate bound (~4-5ns/desc at 4 queues, size-independent),
so v2.1 minimizes descriptor COUNT beyond the one-desc-per-edge gather:
- Self-loops never gathered: own-shard h1p/t2 terms are bulk-loaded p-major
  (~128 descs) and added on DVE.
- Per-pass partials scatter-add (f32, dense 256B rows) into a p-major
  accumulator keyed by true destination, so the epilogue re-reads the whole
  accumulator with 128 descriptors (one contiguous run per partition).
- All launch inputs/outputs that are per-destination use the plain p-major
  [128, 98*F] layout (dst = n*128 + p at [p, n*F:(n+1)*F]) — bulk DMA.
Layer tables as in v2: layer-1 bf16 pair-packed rows (128B descs via the
256B-stride sub-row gather), layer-2 f32 2-wide (8B descs). Gather perms are
per-pass global degree sorts (tight ~5% slot padding).
"""
import numpy as np

N = 100000
E_CH = 128
HID = 64
OUT = 2
NC = 8
PERCORE = 12500
PC_PAD = 12544
NCOLS = 98               # p-major columns (dst = n*128+p, n in [0,98))
NPASS = 4
L1_ROWS = 25001
L2_ROWS = 25001
ZROW = 25000
MAXPOS = 16384           # gather positions per call (slot-major k-blocks);
                         # stage tile is [128, (MAXPOS//128)*F]
STAGE_COLS = MAXPOS // 128


def _wrap_idx(vals):
    ni = len(vals)
    assert ni % 16 == 0
    return np.tile(vals.reshape(ni // 16, 16).T, (8, 1))


def _prep_layer(src, dst, pass_of, row_of, split2=False):
    """Per-layer prep: per-pass degree-sorted perms, per-tile shared-K slot
    matrices, slot-major (k-block) gather calls, gather idx, and p-major
    scatter idx. No self-loops here.

    Slot-major: positions are ordered (k, tile, p) with c_k = #tiles whose
    K exceeds k. Each call covers consecutive k's with sum(c_k)*128 <=
    MAXPOS, so slot padding is per-tile-max only (~3%) and the reduce is a
    prefix-add tree on DVE (packed bf16/f32, 2x mode) instead of a strided
    tensor_reduce."""
    p = pass_of(src)
    r = row_of(src)
    core = dst // PERCORE
    dloc = dst % PERCORE

    cnt = np.bincount(dst * NPASS + p, minlength=N * NPASS).reshape(N, NPASS)

    perms = np.zeros((NC, NPASS, PC_PAD), np.int64)
    K = np.zeros((NPASS, PC_PAD // 128), np.int64)
    for i in range(NC):
        c0 = cnt[i * PERCORE:(i + 1) * PERCORE]
        for q in range(NPASS):
            pp = np.argsort(-c0[:, q], kind="stable")
            pp = np.concatenate([pp, np.arange(PERCORE, PC_PAD)])
            perms[i, q] = pp
            cc = np.concatenate([c0[:, q], np.zeros(PC_PAD - PERCORE, np.int64)])
            kt = cc[pp].reshape(-1, 128).max(axis=1)
            K[q] = np.maximum(K[q], kt)
    K = np.maximum(K, 1)

    invperms = np.zeros((NC, NPASS, PC_PAD), np.int64)
    for i in range(NC):
        for q in range(NPASS):
            invperms[i, q][perms[i, q]] = np.arange(PC_PAD)
    Ms = [[None] * NPASS for _ in range(NC)]
    for i in range(NC):
        ec = core == i
        for q in range(NPASS):
            sel = ec & (p == q)
            ed = dloc[sel]
            es = r[sel]
            order = np.argsort(ed, kind="stable")
            ed = ed[order]
            es = es[order]
            starts = np.searchsorted(ed, np.arange(PERCORE))
            rank = np.arange(len(ed)) - starts[ed]
            kmax = int(K[q].max())
            M = np.full((PC_PAD, kmax), ZROW, np.int16)
            M[invperms[i, q][ed], rank] = es.astype(np.int16)
            Ms[i][q] = M

    # slot-major call packing: per pass, greedy k-blocks under MAXPOS
    calls = []  # (pass, [c_k, ...], k0, tile0)
    for q in range(NPASS):
        kmax = int(K[q].max())
        cs_all = [int((K[q] > k).sum()) for k in range(kmax)]
        cur = []
        k0 = 0
        for ck in cs_all:
            if cur and (sum(cur) + ck) * 128 > MAXPOS:
                calls.append((q, cur, k0, 0))
                k0 += len(cur)
                cur = []
            cur.append(ck)
        if cur:
            calls.append((q, cur, k0, 0))
    # halve the very first call: its descgen (~5.3us) gates the launch ramp
    q0, cs0, _, _ = calls[0]
    if len(cs0) == 1 and cs0[0] > 2:
        a = cs0[0] // 3
        b = 2 * (cs0[0] // 3)
        calls[0:1] = [(q0, [a], 0, 0), (q0, [b - a], 0, a),
                      (q0, [cs0[0] - b], 0, b)]
    # also halve the second k-block of pass 0 (L1 only: its 9us transfers
    # leave descgen slack; L2's floor-rate transfers don't)
    q1, cs1, k1, t1 = calls[3]
    if split2 and q1 == q0 and len(cs1) == 1 and cs1[0] > 1 and t1 == 0:
        h = cs1[0] // 2
        calls[3:4] = [(q1, [h], k1, 0), (q1, [cs1[0] - h], k1, h)]

    idxg = []
    for i in range(NC):
        parts = []
        for (q, cs, k0, t0) in calls:
            vals = np.concatenate(
                [Ms[i][q][t0 * 128:(t0 + ck) * 128, k0 + j].reshape(ck, 128)
                 for j, ck in enumerate(cs)], axis=0)
            parts.append(_wrap_idx(vals.ravel()))
        idxg.append(np.concatenate(parts, axis=1))
    idxg = np.stack(idxg)  # [NC, 128, COLS_G]

    # scatter idx per (core, pass): perm position i -> p-major acc row of the
    # true dst; pad positions (perm rank >= PERCORE, trailing) -> -1
    idxsc = np.zeros((NC, NPASS, 128, PC_PAD // 16), np.int16)
    for i in range(NC):
        for q in range(NPASS):
            d = perms[i, q]
            v = ((d % 128) * NCOLS + d // 128).astype(np.int16)
            v[PERCORE:] = -1
            idxsc[i, q] = _wrap_idx(v)

    ndesc = int(K.sum()) * 128
    return calls, idxg, idxsc, ndesc


def _host_prep(edge_index):
    src = np.asarray(edge_index[0], dtype=np.int64)
    dst = np.asarray(edge_index[1], dtype=np.int64)
    deg = np.bincount(dst, minlength=N).astype(np.float64) + 1.0
    dis = (1.0 / np.sqrt(deg)).astype(np.float32)

    # no appended self-loops; added directly in the epilogues
    l1 = _prep_layer(src, dst,
                     lambda s: (s // 50000) * 2 + (s % 2),
                     lambda s: (s % 50000) // 2, split2=True)
    l2 = _prep_layer(src, dst,
                     lambda s: s // 25000,
                     lambda s: s % 25000)
    return dis, l1, l2


def _bass_mods():
    import sys
    if "/opt/trn_rl_repo" not in sys.path:
        sys.path.insert(0, "/opt/trn_rl_repo")
    import concourse.bass as bass
    import concourse.bacc as bacc
    import concourse.tile as tile
    from concourse import mybir
    from concourse.bass_utils import run_bass_kernel_spmd
    return bass, bacc, tile, mybir, run_bass_kernel_spmd


def _dma_gather_thin(gp, out_ap, in_ap, idxs_ap, num_idxs, elem_size,
                     elem_step, queue_num):
    from concourse import mybir
    gp._assert_queue_num(queue_num)
    assert idxs_ap.dtype == mybir.dt.int16
    stride_bytes = elem_step * mybir.dt.size(in_ap.dtype)
    assert stride_bytes % 256 == 0 and stride_bytes // 256 < 256
    assert in_ap.ap[-1][1] == elem_size
    assert in_ap.ap[0][0] == elem_step
    _in_ap = gp.lower_ap_dma(in_ap, for_custom_bir_dma=True)
    _idxs_ap = gp.lower_ap(idxs_ap)
    _out_ap = gp.lower_ap(out_ap)
    return gp.add_instruction(
        mybir.InstDMAGatherAnt(
            name=gp.bass.get_next_instruction_name(),
            ins=[*_in_ap, _idxs_ap, gp.lower_val_access(gp.to_reg(num_idxs))],
            outs=[_out_ap],
            transpose=False,
            num_idxs=num_idxs,
            elem_size=elem_size,
            stride_bytes_256=stride_bytes // 256,
            gen_mode=0,
            single_packet=False,
            queue_num=queue_num,
            sbuf_tokens_per_rank=0,
            sbuf_free_dim_per_rank=0,
            sbuf_free_dim_pad_per_rank=0,
            sbuf_byte_offset=0,
        )
    )


def _build_mm():
    """h1p = (x @ W1) * dis for own shard, bf16, p-major output."""
    bass, bacc, tile, mybir, _ = _bass_mods()
    from contextlib import ExitStack
    nc = bacc.Bacc()
    bf = mybir.dt.bfloat16
    xT = nc.declare_dram_parameter("xT", [E_CH, PC_PAD], bf, isOutput=False)
    W1 = nc.declare_dram_parameter("W1", [E_CH, HID], bf, isOutput=False)
    disp = nc.declare_dram_parameter("disp", [128, NCOLS], mybir.dt.float32,
                                     isOutput=False)
    out = nc.declare_dram_parameter("out", [128, NCOLS * HID], bf,
                                    isOutput=True)
    G = 14
    with tile.TileContext(nc) as tc, ExitStack() as ctx:
        wp = ctx.enter_context(tc.tile_pool(name="wp", bufs=1))
        sb = ctx.enter_context(tc.tile_pool(name="sb", bufs=3))
        ps = ctx.enter_context(tc.tile_pool(name="ps", bufs=4, space="PSUM"))
        w1 = wp.tile([E_CH, HID], bf, tag="w1")
        nc.sync.dma_start(out=w1[:], in_=W1[:, :])
        dis_sb = wp.tile([128, NCOLS], mybir.dt.float32, tag="dis")
        nc.sync.dma_start(out=dis_sb[:], in_=disp[:, :])
        PB = 7
        for g in range(0, NCOLS, G):
            ng = min(G, NCOLS - g)
            xt = sb.tile([E_CH, G * 128], bf, tag="xt")
            nc.sync.dma_start(out=xt[:, :ng * 128],
                              in_=xT[:, g * 128:(g + ng) * 128])
            ot = sb.tile([128, G * HID], bf, tag="ot")
            for h0 in range(0, ng, PB):
                nh = min(PB, ng - h0)
                pt = ps.tile([128, PB * HID], mybir.dt.float32, space="PSUM",
                             tag="pt")
                for j in range(nh):
                    nc.tensor.matmul(pt[:, j * HID:(j + 1) * HID],
                                     lhsT=xt[:, (h0 + j) * 128:
                                             (h0 + j + 1) * 128],
                                     rhs=w1[:], start=True, stop=True)
                # scale each column's HID block by its dis in one strided op
                dview = bass.AP(dis_sb.tensor, dis_sb[:].offset + g + h0,
                                [dis_sb[:].ap[0], [1, nh], [0, HID]])
                pv = bass.AP(pt.tensor, pt[:].offset,
                             [pt[:].ap[0], [HID, nh], [1, HID]])
                ov = bass.AP(ot.tensor, ot[:].offset + h0 * HID,
                             [ot[:].ap[0], [HID, nh], [1, HID]])
                with nc.allow_low_precision(reason="bf16 h1 staging"):
                    nc.vector.tensor_tensor(out=ov, in0=pv, in1=dview,
                                            op=mybir.AluOpType.mult)
            nc.sync.dma_start(out=out[:, g * HID:(g + ng) * HID],
                              in_=ot[:, :ng * HID])
    nc.compile()
    return nc


def _common_agg(nc, bass, tile, mybir, ctx, tc, calls, tabs, idx0g, idxh,
                repb, idxsc, F, stage_dt, gather_elem, gather_step,
                tab_col_of, acc, acc_step, NQ, split_scatter=True,
                no_scatter=False, no_reduce=False, astrip_bufs=2,
                stage_bufs=4):
    """Shared gather/reduce/scatter pipeline. astrip is compact
    [128, NCOLS*F] in stage_dt; scatter writes F elems per destination into
    `acc` (row stride acc_step elems = 256B; untouched columns stay zero via
    output zero-donation).

    Gather indices arrive as hi/lo bf16 [32, cols] (idxh) and are broadcast
    to the q7-required 8x-replicated int16 [128, cols] layout on-chip:
    PE matmul against repb (256*rep | rep) then an exact f32->int16 convert
    on DVE. This cuts idx HBM traffic 4x. Call 0 uses a small direct int16
    load (idx0g) so the first gather isn't gated on the broadcast pipeline."""
    ib = ctx.enter_context(tc.tile_pool(name="ib", bufs=2))
    stp = ctx.enter_context(tc.tile_pool(name="stp", bufs=stage_bufs))
    ap_ = ctx.enter_context(tc.tile_pool(name="ap", bufs=astrip_bufs))
    psp = ctx.enter_context(tc.tile_pool(name="psp", bufs=2, space="PSUM"))

    cst_local = ctx.enter_context(tc.tile_pool(name="cstl", bufs=1))
    SCC = PC_PAD // 16
    iscb = cst_local.tile([128, NPASS * SCC], mybir.dt.int16, tag="iscb")
    iscb_loaded = [False]
    repb_sb = cst_local.tile([32, 128], mybir.dt.bfloat16, tag="repb")
    nc.sync.dma_start(out=repb_sb[:], in_=repb[:, :])
    BCH = 512  # psum-chunk columns per broadcast matmul

    def ensure_iscb():
        # deferred so the launch ramp isn't spent on scatter indices
        if not iscb_loaded[0]:
            nc.sync.dma_start(out=iscb[:], in_=idxsc[:, :])
            iscb_loaded[0] = True

    qn = 0
    goff = 0
    cur_pass = -1
    idx_sb = None
    idx0_sb = None
    pass_goff = 0
    astrip = None
    pass_cols = {}
    pass_ncalls = {}
    for (q, cs, _k0, _t0) in calls:
        pass_cols[q] = pass_cols.get(q, 0) + sum(cs) * 8
        pass_ncalls[q] = pass_ncalls.get(q, 0) + 1

    HCOL = NCOLS // 2          # 49 astrip columns per scatter half
    HPOS = HCOL * 128          # 6272 positions per half

    # per pass: index (within the pass) of the last call touching any tile
    # >= HCOL; after it, astrip cols [HCOL, NCOLS) are final (c_k shrinks)
    last_big = {}
    seen = {}
    for (q, cs, k0c, t0) in calls:
        j = seen.get(q, 0)
        if t0 + cs[0] > HCOL:
            last_big[q] = j
        seen[q] = j + 1

    def flush_half(q, astrip_t, half):
        if no_scatter:
            return
        ensure_iscb()
        base = astrip_t[:]
        if not split_scatter and half == 1:
            nc.gpsimd.dma_scatter_add(
                out_ap=acc[:, :F],
                in_ap=astrip_t[:].rearrange("p (k f) -> p k f", k=NCOLS),
                idxs_ap=iscb[:, q * SCC:(q + 1) * SCC],
                num_idxs=PC_PAD, num_idxs_reg=PERCORE,
                elem_size=F, elem_step=acc_step,
                queue_num=q % NQ, single_packet=False)
            return
        nc.gpsimd.dma_scatter_add(
            out_ap=acc[:, :F],
            in_ap=bass.AP(astrip_t.tensor, base.offset + half * HCOL * F,
                          [base.ap[0], [F, HCOL], [1, F]]),
            idxs_ap=iscb[:, q * SCC + half * (HPOS // 16):
                         q * SCC + (half + 1) * (HPOS // 16)],
            num_idxs=HPOS,
            num_idxs_reg=HPOS if half == 0 else PERCORE - HPOS,
            elem_size=F, elem_step=acc_step,
            queue_num=q % NQ, single_packet=False)

    # call-0 fast path: direct int16 load so gather 0 isn't gated on the
    # broadcast pipeline (its ~9us transfer then covers the convert latency)
    c0 = sum(calls[0][1]) * 8
    idx0_sb = cst_local.tile([128, c0], mybir.dt.int16, tag="idx0")
    nc.sync.dma_start(out=idx0_sb[:], in_=idx0g[:, :c0])

    # broadcast pipelines are emitted lookahead-1: pass q+1's converts land
    # on DVE between pass q's early reduce ops, so they neither stall the
    # next pass's gathers nor push the whole reduce/scatter chain late
    pass_off = [0] * NPASS
    go = 0
    for q in range(NPASS):
        pass_off[q] = go
        go += pass_cols[q]
    idx_tiles = [None] * NPASS

    def emit_idx_pipeline(q):
        ccols = pass_cols[q]
        idxh_sb = ib.tile([32, ccols], mybir.dt.bfloat16, tag="idxh")
        nc.sync.dma_start(out=idxh_sb[:],
                          in_=idxh[:, pass_off[q]:pass_off[q] + ccols])
        idx_sb = cst_local.tile([128, ccols], mybir.dt.int16, tag=f"idx{q}")
        for o in range(0, ccols, BCH):
            w = min(BCH, ccols - o)
            pidx = psp.tile([128, BCH], mybir.dt.float32, space="PSUM",
                            tag="pidx")
            nc.tensor.matmul(pidx[:, :w], lhsT=repb_sb[:],
                             rhs=idxh_sb[:, o:o + w],
                             start=True, stop=True)
            nc.vector.tensor_scalar_add(idx_sb[:, o:o + w],
                                        pidx[:, :w], 0.0)
        idx_tiles[q] = idx_sb

    emit_idx_pipeline(0)

    call_in_pass = 0
    for (q, cs, k0c, t0) in calls:
        if q != cur_pass:
            if astrip is not None:
                # high-degree half (cols [0, HCOL)) finalizes at pass end
                flush_half(cur_pass, astrip,
                           0 if split_scatter else 1)
            cur_pass = q
            pass_goff = goff
            call_in_pass = 0
            if idx_tiles[q] is None:
                emit_idx_pipeline(q)
            idx_sb = idx_tiles[q]
            astrip = ap_.tile([128, NCOLS * F], stage_dt, tag="astrip")
        tot = sum(cs)
        ni = tot * 128
        stage = stp.tile([128, STAGE_COLS * F], stage_dt, tag="stage")
        lo = goff - pass_goff
        if q == 0 and lo == 0:
            idx_view = idx0_sb[:, :tot * 8]
        else:
            idx_view = idx_sb[:, lo:lo + tot * 8]
        _dma_gather_thin(
            nc.gpsimd,
            out_ap=bass.AP(stage.tensor, stage[:].offset,
                           [stage[:].ap[0], [F, tot], [1, F]]),
            in_ap=tab_col_of(q),
            idxs_ap=idx_view,
            num_idxs=ni, elem_size=gather_elem, elem_step=gather_step,
            queue_num=qn)
        qn = (qn + 1) % NQ

        # prefix-add tree over the call's k-blocks (c nonincreasing), then
        # one add (or init copy) into astrip[0 : c_first*F)
        if no_reduce:
            goff += tot * 8
            call_in_pass += 1
            continue
        sap0 = stage[:].ap[0]
        soff = stage[:].offset
        blocks = []
        o = 0
        for ck in cs:
            blocks.append((o, ck))
            o += ck
        with nc.allow_low_precision(reason="short partial sums, tree depth"):
            while len(blocks) > 1:
                nxt = []
                for a in range(0, len(blocks) - 1, 2):
                    (o0, c0b), (o1, c1b) = blocks[a], blocks[a + 1]
                    v0 = bass.AP(stage.tensor, soff + o0 * F,
                                 [sap0, [F, c1b], [1, F]])
                    v1 = bass.AP(stage.tensor, soff + o1 * F,
                                 [sap0, [F, c1b], [1, F]])
                    nc.vector.tensor_tensor(out=v0, in0=v0, in1=v1,
                                            op=mybir.AluOpType.add)
                    nxt.append((o0, c0b))
                if len(blocks) % 2:
                    nxt.append(blocks[-1])
                blocks = nxt
            (o0, cfin) = blocks[0]
            srcap = bass.AP(stage.tensor, soff + o0 * F,
                            [sap0, [F, cfin], [1, F]])
            dstap = bass.AP(astrip.tensor, astrip[:].offset + t0 * F,
                            [astrip[:].ap[0], [F, cfin], [1, F]])
            if call_in_pass == 0 or (t0 > 0 and k0c == 0):
                # c_0 == NCOLS (K >= 1 everywhere): initializes all of astrip
                nc.vector.tensor_scalar_add(out=dstap, in0=srcap, scalar1=0.0)
            else:
                nc.vector.tensor_tensor(out=dstap, in0=dstap, in1=srcap,
                                        op=mybir.AluOpType.add)
        goff += tot * 8
        if split_scatter and call_in_pass == last_big[q]:
            # low-degree half's tiles are never touched by later (smaller-c)
            # calls of this pass
            flush_half(q, astrip, 1)
        if call_in_pass == 1 and q + 1 < NPASS and idx_tiles[q + 1] is None:
            emit_idx_pipeline(q + 1)
        call_in_pass += 1
    flush_half(cur_pass, astrip, 0 if split_scatter else 1)


def _build_agg1(calls, cols_g, skip_epi=False, no_scatter=False,
                no_reduce=False):
    """Layer-1 aggregation + self add + epilogue t2 = relu(...) @ W2."""
    bass, bacc, tile, mybir, _ = _bass_mods()
    from contextlib import ExitStack
    from concourse.masks import make_identity
    bf = mybir.dt.bfloat16
    f32 = mybir.dt.float32
    NQ = 4
    nc = bacc.Bacc(num_swdge_queues=NQ, dynamic_dma_scratch_size=8192 * NQ)
    tabs = [nc.declare_dram_parameter(f"tab{c}", [L1_ROWS, 128], bf,
                                      isOutput=False) for c in range(2)]
    c0 = sum(calls[0][1]) * 8
    idx0g = nc.declare_dram_parameter("idx0g", [128, c0], mybir.dt.int16,
                                      isOutput=False)
    idxh = nc.declare_dram_parameter("idxh", [32, cols_g], bf, isOutput=False)
    repb = nc.declare_dram_parameter("repb", [32, 128], bf, isOutput=False)
    idxsc = nc.declare_dram_parameter("idxsc", [128, NPASS * (PC_PAD // 16)],
                                      mybir.dt.int16, isOutput=False)
    disp = nc.declare_dram_parameter("disp", [128, NCOLS], f32, isOutput=False)
    selfh = nc.declare_dram_parameter("selfh", [128, NCOLS * HID], bf,
                                      isOutput=False)
    W2 = nc.declare_dram_parameter("W2", [HID, OUT], bf, isOutput=False)
    acc = nc.declare_dram_parameter("acc", [PC_PAD, 128], bf, isOutput=True)
    out = nc.declare_dram_parameter("out", [128, NCOLS * OUT], f32,
                                    isOutput=True)

    with tile.TileContext(nc) as tc, ExitStack() as ctx:
        cst = ctx.enter_context(tc.tile_pool(name="cst", bufs=1))
        ep = ctx.enter_context(tc.tile_pool(name="ep", bufs=3))
        ps = ctx.enter_context(tc.tile_pool(name="ps", bufs=4, space="PSUM"))

        dis_sb = cst.tile([128, NCOLS], f32, tag="dis")
        nc.sync.dma_start(out=dis_sb[:], in_=disp[:, :])
        w2t = cst.tile([HID, OUT], bf, tag="w2t")
        nc.sync.dma_start(out=w2t[:], in_=W2[:, :])
        ident = cst.tile([128, 128], bf, tag="ident")
        make_identity(nc, ident[:])

        _common_agg(nc, bass, tile, mybir, ctx, tc, calls, tabs, idx0g, idxh,
                    repb, idxsc,
                    F=HID, stage_dt=bf, gather_elem=HID, gather_step=128,
                    tab_col_of=lambda q: tabs[q // 2][:, (q % 2) * HID:
                                                      (q % 2 + 1) * HID],
                    acc=acc, acc_step=128, NQ=NQ,
                    no_scatter=no_scatter, no_reduce=no_reduce)

        # ---- epilogue (GE-chunked reads of the p-major bf16 accumulator;
        # acc rows are 128-wide with cols HID..128 zero from donation).
        # selfh comes in with b1/dis pre-folded on host, so
        # a1 = relu(dis^2 * (S + selfh)); PSUM work is batched PB columns per
        # ACT copy to amortize the ~370ns scalar-engine access latency. ----
        GE = 10
        PB = 5
        for g0 in ([] if skip_epi else range(0, NCOLS, GE)):
            ng = min(GE, NCOLS - g0)
            sS = ep.tile([128, GE * 128], bf, tag="sS")
            accb = acc[:, :]
            nc.sync.dma_start(
                out=sS[:, :ng * 128].rearrange("p (m f) -> p m f", m=ng),
                in_=bass.AP(accb.tensor, accb.offset + g0 * 128,
                            [[NCOLS * 128, 128], [128, ng], [1, 128]]))
            selft = ep.tile([128, GE * HID], bf, tag="selft")
            nc.sync.dma_start(out=selft[:, :ng * HID],
                              in_=selfh[:, g0 * HID:(g0 + ng) * HID])
            svs = bass.AP(sS.tensor, sS[:].offset,
                          [sS[:].ap[0], [128, ng], [1, HID]])
            selfv = bass.AP(selft.tensor, selft[:].offset,
                            [selft[:].ap[0], [HID, ng], [1, HID]])
            with nc.allow_low_precision(reason="bf16 self add"):
                nc.vector.tensor_tensor(out=svs, in0=svs, in1=selfv,
                                        op=mybir.AluOpType.add)
            a1 = ep.tile([128, GE * HID], bf, tag="a1")
            dview = bass.AP(dis_sb.tensor, dis_sb[:].offset + g0,
                            [dis_sb[:].ap[0], [1, ng], [0, HID]])
            sv = bass.AP(sS.tensor, sS[:].offset,
                         [sS[:].ap[0], [128, ng], [1, HID]])
            av = bass.AP(a1.tensor, a1[:].offset,
                         [a1[:].ap[0], [HID, ng], [1, HID]])
            with nc.allow_low_precision(reason="bf16 epilogue"):
                nc.vector.tensor_tensor(out=av, in0=sv, in1=dview,
                                        op=mybir.AluOpType.mult)
                nc.vector.tensor_scalar_max(a1[:, :ng * HID],
                                            a1[:, :ng * HID], 0.0)
            ostrip = ep.tile([128, GE * OUT], f32, tag="ostrip")
            for h0 in range(0, ng, PB):
                nh = min(PB, ng - h0)
                putb = ps.tile([HID, PB * 128], bf, space="PSUM", tag="putb")
                for j in range(nh):
                    nc.tensor.transpose(
                        out=putb[:, j * 128:(j + 1) * 128],
                        in_=a1[:, (h0 + j) * HID:(h0 + j + 1) * HID],
                        identity=ident[:])
                utb = ep.tile([HID, PB * 128], bf, tag="utb")
                nc.scalar.activation(out=utb[:, :nh * 128],
                                     in_=putb[:, :nh * 128],
                                     func=mybir.ActivationFunctionType.Copy)
                pob = ps.tile([128, PB * OUT], f32, space="PSUM", tag="pob")
                for j in range(nh):
                    nc.tensor.matmul(pob[:, j * OUT:(j + 1) * OUT],
                                     lhsT=utb[:, j * 128:(j + 1) * 128],
                                     rhs=w2t[:], start=True, stop=True)
                nc.scalar.activation(out=ostrip[:, h0 * OUT:(h0 + nh) * OUT],
                                     in_=pob[:, :nh * OUT],
                                     func=mybir.ActivationFunctionType.Copy)
            nc.sync.dma_start(out=out[:, g0 * OUT:(g0 + ng) * OUT],
                              in_=ostrip[:, :ng * OUT])
    nc.compile()
    return nc


def _build_agg2(calls, cols_g, skip_epi=False, no_scatter=False,
                no_reduce=False):
    """Layer-2 aggregation of 2-wide f32 + self add + S2*dis + b2."""
    bass, bacc, tile, mybir, _ = _bass_mods()
    from contextlib import ExitStack
    f32 = mybir.dt.float32
    NQ = 4
    nc = bacc.Bacc(num_swdge_queues=NQ, dynamic_dma_scratch_size=8192 * NQ)
    tabs = [nc.declare_dram_parameter(f"tab{c}", [L2_ROWS, 64], f32,
                                      isOutput=False) for c in range(NPASS)]
    bf = mybir.dt.bfloat16
    c0 = sum(calls[0][1]) * 8
    idx0g = nc.declare_dram_parameter("idx0g", [128, c0], mybir.dt.int16,
                                      isOutput=False)
    idxh = nc.declare_dram_parameter("idxh", [32, cols_g], bf, isOutput=False)
    repb = nc.declare_dram_parameter("repb", [32, 128], bf, isOutput=False)
    idxsc = nc.declare_dram_parameter("idxsc", [128, NPASS * (PC_PAD // 16)],
                                      mybir.dt.int16, isOutput=False)
    disp = nc.declare_dram_parameter("disp", [128, NCOLS], f32, isOutput=False)
    b2b = nc.declare_dram_parameter("b2b", [128, OUT], f32, isOutput=False)
    selft2 = nc.declare_dram_parameter("selft2", [128, NCOLS * OUT], f32,
                                       isOutput=False)
    acc = nc.declare_dram_parameter("acc", [PC_PAD, 64], f32, isOutput=True)
    out = nc.declare_dram_parameter("out", [128, NCOLS * OUT], f32,
                                    isOutput=True)

    with tile.TileContext(nc) as tc, ExitStack() as ctx:
        cst = ctx.enter_context(tc.tile_pool(name="cst", bufs=1))
        big = ctx.enter_context(tc.tile_pool(name="big", bufs=1))

        dis_sb = cst.tile([128, NCOLS], f32, tag="dis")
        nc.sync.dma_start(out=dis_sb[:], in_=disp[:, :])
        b2t = cst.tile([128, OUT], f32, tag="b2t")
        nc.sync.dma_start(out=b2t[:], in_=b2b[:, :])

        _common_agg(nc, bass, tile, mybir, ctx, tc, calls, tabs, idx0g, idxh,
                    repb, idxsc,
                    F=OUT, stage_dt=f32, gather_elem=OUT, gather_step=64,
                    tab_col_of=lambda q: tabs[q][:, :OUT],
                    acc=acc, acc_step=64, NQ=NQ, split_scatter=True,
                    no_scatter=no_scatter, no_reduce=no_reduce,
                    astrip_bufs=4, stage_bufs=8)

        # ---- epilogue: out = (S2 + self)*dis + b2; read only the 2 used
        # f32 of each 64-wide acc row (8B strided elems ride the 7ns floor)
        if skip_epi:
            nc.compile()
            return nc
        sS = big.tile([128, NCOLS * OUT], f32, tag="sS")
        accb = acc[:, :]
        nc.sync.dma_start(
            out=sS[:].rearrange("p (m f) -> p m f", m=NCOLS),
            in_=bass.AP(accb.tensor, accb.offset,
                        [[NCOLS * 64, 128], [64, NCOLS], [1, OUT]]))
        selft = big.tile([128, NCOLS * OUT], f32, tag="selft")
        nc.sync.dma_start(out=selft[:], in_=selft2[:, :])
        nc.vector.tensor_tensor(
            out=sS[:].rearrange("p (m f) -> p m f", m=NCOLS),
            in0=sS[:].rearrange("p (m f) -> p m f", m=NCOLS),
            in1=selft[:].rearrange("p (m f) -> p m f", m=NCOLS),
            op=mybir.AluOpType.add)
        dview = bass.AP(dis_sb.tensor, dis_sb[:].offset,
                        [dis_sb[:].ap[0], [1, NCOLS], [0, OUT]])
        sv = bass.AP(sS.tensor, sS[:].offset,
                     [sS[:].ap[0], [OUT, NCOLS], [1, OUT]])
        b2view = bass.AP(b2t.tensor, b2t[:].offset,
                         [b2t[:].ap[0], [0, NCOLS], [1, OUT]])
        nc.vector.tensor_tensor(out=sv, in0=sv, in1=dview,
                                op=mybir.AluOpType.mult)
        nc.vector.tensor_tensor(out=sv, in0=sv, in1=b2view,
                                op=mybir.AluOpType.add)
        nc.sync.dma_start(out=out[:, :], in_=sS[:])
    nc.compile()
    return nc


def _pmajor(arr_pad):
    """[PC_PAD, F] node order -> [128, NCOLS*F] p-major."""
    F = arr_pad.shape[1]
    return np.ascontiguousarray(
        arr_pad.reshape(NCOLS, 128, F).transpose(1, 0, 2).reshape(128, NCOLS * F))


def _unpmajor(arr_pm, F):
    """[128, NCOLS*F] p-major -> [PC_PAD, F] node order."""
    return np.ascontiguousarray(
        arr_pm.reshape(128, NCOLS, F).transpose(1, 0, 2).reshape(PC_PAD, F))


def kernel(x, edge_index, W1, b1, W2, b2):
    import ml_dtypes
    bf16 = ml_dtypes.bfloat16
    x = np.asarray(x, dtype=np.float32)
    W1 = np.asarray(W1, dtype=np.float32)
    b1 = np.asarray(b1, dtype=np.float32)
    W2 = np.asarray(W2, dtype=np.float32)
    b2 = np.asarray(b2, dtype=np.float32)

    bass, bacc, tile, mybir, run_spmd = _bass_mods()

    dis, (c1, x1, s1, nd1), (c2, x2, s2, nd2) = _host_prep(edge_index)
    cores = list(range(NC))

    # idx broadcast operands: hi/lo bf16 rows of the 16-partition wrap, and
    # the stacked replication matrix (256*rep | rep)
    def _idx_ops(xg, calls):
        base = xg[:, :16, :].astype(np.int32)   # [NC, 16, cols]
        idxh = np.concatenate([base // 256, base % 256], axis=1).astype(bf16)
        c0 = sum(calls[0][1]) * 8
        idx0g = np.ascontiguousarray(xg[:, :, :c0])
        return idxh, idx0g

    repb = np.zeros((32, 128), bf16)
    for p in range(128):
        repb[p % 16, p] = 256.0
        repb[16 + p % 16, p] = 1.0

    def _dpad(i):
        dp = np.concatenate([dis[i * PERCORE:(i + 1) * PERCORE],
                             np.ones(PC_PAD - PERCORE, np.float32)])
        return dp

    disps = [np.ascontiguousarray(_dpad(i).reshape(NCOLS, 128).T)
             for i in cores]

    # ---- launch 1: mm ----
    nc1 = _build_mm()
    in1 = []
    for i in cores:
        xT = np.zeros((E_CH, PC_PAD), bf16)
        xT[:, :PERCORE] = x[i * PERCORE:(i + 1) * PERCORE].T.astype(bf16)
        in1.append({"xT": xT, "W1": W1.astype(bf16), "disp": disps[i]})
    r1 = run_spmd(nc1, in1, core_ids=cores)
    h1p = np.concatenate([
        _unpmajor(np.asarray(r1.results[i]["out"]), HID)[:PERCORE]
        for i in cores])  # [N, HID] bf16

    # ---- host: pack layer-1 pair tables ----
    tabs1 = []
    for c in range(2):
        t = np.zeros((L1_ROWS, 128), bf16)
        t[:25000] = h1p[c * 50000:(c + 1) * 50000].reshape(25000, 128)
        tabs1.append(t)

    # ---- launch 2 ----
    nc2 = _build_agg1(c1, x1.shape[2])
    idxh1, idx0g1 = _idx_ops(x1, c1)
    in2 = []
    for i in cores:
        dp = np.concatenate([dis[i * PERCORE:(i + 1) * PERCORE],
                             np.ones(PC_PAD - PERCORE, np.float32)])
        # fold the bias in: a1 = relu(dis^2*(S + selfh + b1/dis)) on device
        h1pad = np.zeros((PC_PAD, HID), np.float32)
        h1pad[:PERCORE] = h1p[i * PERCORE:(i + 1) * PERCORE].astype(np.float32)
        h1pad += b1[None, :] / dp[:, None]
        m = {f"tab{c}": tabs1[c] for c in range(2)}
        m.update({
            "idxh": idxh1[i],
            "idx0g": idx0g1[i],
            "repb": repb,
            "idxsc": np.concatenate([s1[i, q] for q in range(NPASS)], axis=1),
            # epilogue constant: dis^2 per destination
            "disp": np.ascontiguousarray((dp * dp).reshape(NCOLS, 128).T),
            "selfh": _pmajor(h1pad.astype(bf16)),
            "W2": W2.astype(bf16),
        })
        in2.append(m)
    r2 = run_spmd(nc2, in2, core_ids=cores)
    t2 = np.concatenate([
        _unpmajor(np.asarray(r2.results[i]["out"]), OUT)[:PERCORE]
        for i in cores])  # [N, 2] f32

    # ---- host: pack layer-2 tables ----
    tabs2 = []
    for c in range(NPASS):
        t = np.zeros((L2_ROWS, 64), np.float32)
        t[:25000, :OUT] = t2[c * 25000:(c + 1) * 25000]
        tabs2.append(t)

    # ---- launch 3 ----
    nc3 = _build_agg2(c2, x2.shape[2])
    idxh2, idx0g2 = _idx_ops(x2, c2)
    b2bc = np.broadcast_to(b2, (128, OUT)).astype(np.float32).copy()
    in3 = []
    for i in cores:
        t2pad = np.zeros((PC_PAD, OUT), np.float32)
        t2pad[:PERCORE] = t2[i * PERCORE:(i + 1) * PERCORE]
        m = {f"tab{c}": tabs2[c] for c in range(NPASS)}
        m.update({
            "idxh": idxh2[i],
            "idx0g": idx0g2[i],
            "repb": repb,
            "idxsc": np.concatenate([s2[i, q] for q in range(NPASS)], axis=1),
            "disp": disps[i],
            "b2b": b2bc,
            "selft2": _pmajor(t2pad),
        })
        in3.append(m)
    r3 = run_spmd(nc3, in3, core_ids=cores)
    outv = np.concatenate([
        _unpmajor(np.asarray(r3.results[i]["out"]), OUT)[:PERCORE]
        for i in cores])
    return outv.astype(np.float32)

